# revision 30
# baseline (speedup 1.0000x reference)
"""Dynamic lightweight convolution TRN2 kernel — banded-matmul design.

out[b,l,d] = (1/K) * sum_k softmax_k(x[b,l+K-1,:] @ W + bias)[k, d%H] * x[b,l+k,d]

B=8, S=2048, D=1024, K=7, H=16, L=S-K+1=2042.
Sharding: data-parallel over batch, one batch element per NeuronCore (8 cores).

Per-core plan — the conv runs on the tensor engine as banded-matrix matmuls:

  1. x is loaded by GPSIMD (SWDGE) casting DMAs: f32 HBM -> bf16 SBUF, in
     graduated batches (chunks 0,1 alone for fast pipeline start, then
     2-chunk/3-chunk batches to amortize ~1us SWDGE descriptor-gen each).
  2. Logits path: PE-transpose x chunks -> xT, logits = W^T @ xT (PE, fp32
     psum), e = exp(logits + bias) (ACT), denominators via a [112,112]
     selector matmul (PE), rinv = 1/. (DVE), en = e * rinv (DVE).  W/bias
     columns are host-permuted k -> 6-k, so row 16j+h of en is the
     normalized weight of tap k = 6-j.
  3. Shifted SBUF->SBUF DMAs build et[16j+h, s] = en[16j+h, s+j] (one DMA
     per j, batched in 3 parts gated by front(1..3)); PE-transposes give
     T[s, r] (r = 16j+h) in t_all.
  4. Band construction via a DRAM bounce (the BIR verifier rejects SBUF
     dst APs with non-partition-exact steps, so the skew must happen on
     DRAM-side strides): T chunk pairs are written to zero-filled DRAM
     images at skewed offsets SKEW*p + r and read back with row pitch
     PITCH = SKEW-16, which lands T[p, r] at band position (p, 16p + r).
     Images are paired (2 tensors x 2 slabs) so one write covers 2 chunks
     and one read covers 2 blocks — halves the per-DMA issue overhead.
  5. Conv for 128-row output block b: for each h, a banded matmul
       out[l, d'] = sum_s A1_h[s, l] * x[s, 16d'+h]
     with stationary A1_h = a1[:, h : h+2048 : 16].  The 6-row contraction
     tail (s in the next chunk) uses a2: a tiny [6, 112] load from the
     next image's left guard + a second matmul into the same psum.
  6. psum [128, 1024] (h-major) -> SBUF staging with a de-interleaving
     copy; two blocks share one staging tile and one paired store DMA.

Queue plan (per-DMA SEQ occupancy ~650ns + head-of-line waits, so queues
are balanced and ordered by data readiness):
  SP (sync):   consts, paired dma1 band writes, paired a1 band reads
  Act (scalar): et shifts, output stores (+ exp, half the ob copies)
  Pool (gpsimd): input loads, zero fills, a2 guard loads (SWDGE)
  DVE (vector): xT copies, t_all copies, recip/mul, half the ob copies
"""

import numpy as np
import ml_dtypes
from contextlib import ExitStack

import concourse.bacc as bacc
import concourse.tile as tile
from concourse import mybir
from concourse import bass_utils
from concourse.ap import AP

K = 7
H = 16
B, S, D = 8, 2048, 1024
L = S - K + 1  # 2042
C = D // 128  # 8 d-chunks
NCH = S // 128  # 16 s-chunks
NB = 16  # output blocks of 128 rows (last has 122 valid)
KH = K * H  # 112

SLOT0 = 96  # img col of (l_rel=0, h=0): band tiles are loaded from this col
ACOLS = 2048  # band-tile cols needed by the stationary views
A2LO = 1952  # a2 col of (l_rel=122, h=0); cols below are zero
A2COLS = 2064  # a2 tile width: loads land at [A2LO, A2LO+112)
IMG_PITCH = 2160  # image read pitch (elements)
IMG_SKEW = IMG_PITCH + 16  # image write pitch: +16 elems (one slot) per row
IMG_FILL = SLOT0 + ACOLS  # 2144: union of read windows per row
IMG_ELEMS = IMG_PITCH * 128  # one slab

F32 = mybir.dt.float32
BF16 = mybir.dt.bfloat16

# byte offsets (per partition) inside the packed constants blob
_OFF_BIAS = 0      # [112, 1] f32
_OFF_IDENTB = 4    # [128, 128] bf16
_OFF_SELSUM = 260  # [112, 112] bf16
_OFF_WT = 484      # [128, 8, 112] bf16
_CONST_BYTES = 2276  # 569 f32 columns


def _host_constants(W, b):
    """Pack bias/identb/selsum/W into one [128, 569] f32 blob."""
    buf = np.zeros((128, _CONST_BYTES), np.uint8)

    def put(off, arr):
        by = np.ascontiguousarray(arr).view(np.uint8).reshape(arr.shape[0], -1)
        buf[: arr.shape[0], off : off + by.shape[1]] = by

    # Permute the k-axis (k -> 6-k) of W and bias so that logits/e/en rows
    # come out in j-order (row 16j+h is the weight for tap k=6-j), matching
    # the band-image run layout r = 16j+h.
    perm = np.array([16 * (K - 1 - j) + h for j in range(K) for h in range(H)])
    put(_OFF_BIAS, np.asarray(b, np.float32)[perm].reshape(KH, 1))
    put(_OFF_IDENTB, np.eye(128).astype(ml_dtypes.bfloat16))
    hh = np.arange(KH) % H
    selsum = ((hh[:, None] == hh[None, :]) * float(K)).astype(ml_dtypes.bfloat16)
    put(_OFF_SELSUM, selsum)
    # W [D, KH] -> permuted -> [128, C, KH] chunks (d = c*128 + p)
    wt = np.asarray(W, np.float32)[:, perm].astype(ml_dtypes.bfloat16)
    wt = wt.reshape(C, 128, KH).transpose(1, 0, 2).reshape(128, C * KH)
    put(_OFF_WT, np.ascontiguousarray(wt))
    return buf.view(np.float32)


def build_program():
    nc = bacc.Bacc(
        "TRN2", target_bir_lowering=False, debug=False, enable_asserts=True
    )

    x_d = nc.dram_tensor("x", [S, D], F32, kind="ExternalInput").ap()
    consts_d = nc.dram_tensor(
        "consts", [128, _CONST_BYTES // 4], F32, kind="ExternalInput"
    ).ap()
    out_d = nc.dram_tensor("out", [L, D], F32, kind="ExternalOutput").ap()
    # two paired images, 2 slabs each (blocks 2q, 2q+1 share one tensor)
    imgp = [
        nc.dram_tensor(f"imgp{i}", [2 * IMG_ELEMS], BF16, kind="Internal").ap()
        for i in range(2)
    ]

    def img_of(b):
        """(tensor ap, element offset) of block b's image slab."""
        return imgp[(b // 2) % 2], (b % 2) * IMG_ELEMS

    with tile.TileContext(nc) as tc, ExitStack() as ctx:
        singles = ctx.enter_context(tc.tile_pool(name="singles", bufs=1))
        xT_pool = ctx.enter_context(tc.tile_pool(name="xT", bufs=3))
        a1_pool = ctx.enter_context(tc.tile_pool(name="a1", bufs=3))
        outs_pool = ctx.enter_context(tc.tile_pool(name="outs", bufs=3))

        p_tp = ctx.enter_context(tc.tile_pool(name="ptp", bufs=3, space="PSUM"))
        p_log = ctx.enter_context(tc.tile_pool(name="plog", bufs=1, space="PSUM"))
        p_out = ctx.enter_context(tc.tile_pool(name="pout", bufs=2, space="PSUM"))

        # ---- constants: one packed DMA, tiles are views into the blob ----
        cblob = singles.tile([128, _CONST_BYTES // 4], F32)
        nc.sync.dma_start(out=cblob, in_=consts_d)
        cbytes = cblob.bitcast(mybir.dt.uint8)

        def cview(off, nbytes, dt, rows=128):
            return cbytes[:rows, off : off + nbytes].bitcast(dt)

        bias_t = cview(_OFF_BIAS, 4, F32, rows=KH)
        identb_t = cview(_OFF_IDENTB, 256, BF16)
        selsum_t = cview(_OFF_SELSUM, 224, BF16, rows=KH)
        wt = cview(_OFF_WT, 1792, BF16).rearrange("p (c n) -> p c n", c=C)

        # GPSIMD ucode warmup — dependency-free so input DMAs start at t~0
        warm = singles.tile([1, 8], BF16)
        nc.gpsimd.memset(warm, 0.0)

        # ---- persistent tensors ----
        xq = [
            singles.tile([128, 4 * D], BF16, name=f"xq{i}") for i in range(4)
        ]

        def xbv(i):
            return xq[i // 4][:, D * (i % 4) : D * (i % 4 + 1)]

        e_full = singles.tile([KH, S], BF16)
        rinv = singles.tile([KH, S], F32)
        en = singles.tile([KH, S], BF16)
        et = singles.tile([KH, S], BF16)  # et[16j+h, s] = en[16j+h, s+j]
        t_all = singles.tile([128, NCH, KH], BF16)  # T[s, r], chunked
        zt = singles.tile([128, IMG_FILL], BF16)  # zeros for image fill
        a2t = [
            singles.tile([6, A2COLS], BF16, name=f"a2t{i}") for i in range(2)
        ]

        # ---- prologue: input DMAs first (gate the whole front pipeline) ----
        # graduated batching: chunks 0,1 alone so front(0) starts ASAP
        for lo, hi in ((0, 1), (1, 2), (2, 4), (4, 6), (6, 8), (8, 10),
                       (10, 12), (12, 16)):
            q, c0 = lo // 4, lo % 4
            assert hi - lo <= 4 and (hi - 1) // 4 == q
            nc.gpsimd.dma_start(
                out=AP(tensor=xq[q][:, :].tensor, offset=c0 * D,
                       ap=[[4 * D, 128], [D, hi - lo], [1, D]]),
                in_=AP(tensor=x_d.tensor, offset=128 * lo * D,
                       ap=[[D, 128], [128 * D, hi - lo], [1, D]]),
            )
        # zero tile for image fills (after input issue — Pool engine order)
        nc.gpsimd.memset(zt, 0.0)

        # ---- stage helpers ----
        def front(sb):
            """Transpose chunks 4sb..4sb+3, logits, exp, denom, rinv, en."""
            sl = slice(512 * sb, 512 * (sb + 1))
            xTt = xT_pool.tile([128, C, 512], BF16, tag="xT")
            for q in range(4):
                i = 4 * sb + q
                xbi = xbv(i)
                ptp = p_tp.tile([128, D], BF16, tag="ptp")
                for c in range(C):
                    nc.tensor.transpose(
                        ptp[:, 128 * c : 128 * (c + 1)],
                        xbi[:, 128 * c : 128 * (c + 1)],
                        identb_t,
                    )
                nc.vector.tensor_copy(
                    xTt[:, :, 128 * q : 128 * (q + 1)],
                    ptp.rearrange("p (c s) -> p c s", c=C),
                )
            plog = p_log.tile([KH, 512], F32, tag="plog")
            for q in range(4):
                for c in range(C):
                    nc.tensor.matmul(
                        plog[:, 128 * q : 128 * (q + 1)],
                        wt[:, c, :],
                        xTt[:, c, 128 * q : 128 * (q + 1)],
                        start=(c == 0), stop=(c == C - 1),
                    )
            nc.scalar.activation(
                e_full[:, sl], plog,
                mybir.ActivationFunctionType.Exp, bias=bias_t, scale=1.0,
            )
            psd = p_log.tile([KH, 512], F32, tag="plog")
            nc.tensor.matmul(psd, selsum_t, e_full[:, sl], start=True, stop=True)
            nc.vector.reciprocal(rinv[:, sl], psd)
            nc.vector.tensor_mul(en[:, sl], e_full[:, sl], rinv[:, sl])

        def shifts_part(part):
            """et[16j+h, s] = en[16j+h, s+j] — one DMA per j, three batches.

            Engine copies can't start at partition 16j (BIR rule: starts must
            be 0/32/64/96) and SBUF DMA APs need partition-exact steps, so:
            plain 2-dim SBUF->SBUF DMAs, one per j-group.  Part p covers the
            et cols ready after front(p+1): [0,512), [512,1024), [1024,2048).
            """
            c0 = 512 * part
            for j in range(K):
                ln = 512 if part < 2 else 1024 - j
                nc.scalar.dma_start(
                    out=AP(tensor=et[:, :].tensor, offset=16 * j * S + c0,
                           ap=[[S, 16], [1, ln]]),
                    in_=AP(tensor=en[:, :].tensor, offset=16 * j * S + c0 + j,
                           ap=[[S, 16], [1, ln]]),
                )

        def t_chunks(lo, hi):
            for i in range(lo, hi):
                pt = p_tp.tile([128, D], BF16, tag="ptp")
                nc.tensor.transpose(
                    pt[:, :KH], et[:, 128 * i : 128 * (i + 1)],
                    identb_t[:KH, :KH],
                )
                nc.vector.tensor_copy(t_all[:, i, :], pt[:, :KH])

        dma1_done = set()

        def dma1p(qq):
            """T chunks 2qq, 2qq+1 -> paired band image (skewed write)."""
            if qq in dma1_done or qq >= NB // 2:
                return
            dma1_done.add(qq)
            im, _ = img_of(2 * qq)
            nc.sync.dma_start(
                out=AP(tensor=im.tensor, offset=0,
                       ap=[[IMG_SKEW, 128], [IMG_ELEMS, 2], [1, KH]]),
                in_=AP(tensor=t_all[:, :, :].tensor, offset=2 * qq * KH,
                       ap=[[NCH * KH, 128], [KH, 2], [1, KH]]),
            )

        dma2s_done = set()

        def dma2s(b):
            """a2 tail for block b: the left-guard cells of block b+1's img."""
            if b in dma2s_done or b + 1 >= NB:
                return
            dma2s_done.add(b)
            im, off = img_of(b + 1)
            nc.gpsimd.dma_start(
                out=a2t[b % 2][:, A2LO : A2LO + KH],
                in_=AP(tensor=im.tensor, offset=off,
                       ap=[[IMG_PITCH, 6], [1, KH]]),
            )

        a1_tiles = {}

        def readp(qq):
            """Paired band read: blocks 2qq, 2qq+1 -> one [128, 2*ACOLS]."""
            if qq in a1_tiles or qq >= NB // 2:
                return
            im, _ = img_of(2 * qq)
            a1 = a1_pool.tile([128, 2 * ACOLS], BF16, tag="a1")
            nc.sync.dma_start(
                out=AP(tensor=a1[:, :].tensor, offset=0,
                       ap=[[2 * ACOLS, 128], [ACOLS, 2], [1, ACOLS]]),
                in_=AP(tensor=im.tensor, offset=SLOT0,
                       ap=[[IMG_PITCH, 128], [IMG_ELEMS, 2], [1, ACOLS]]),
            )
            a1_tiles[qq] = a1

        def block(b, ob, obhalf):
            """Banded conv for output rows 128b .. 128b+nl -> staging tile."""
            readp(b // 2)
            a1 = a1_tiles[b // 2]
            if b % 2 == 1:
                a1_tiles.pop(b // 2)
            acol0 = (b % 2) * ACOLS
            xbb = xbv(b)
            xbn = xbv(b + 1) if b + 1 < NB else None
            po = p_out.tile([128, D], F32, tag="pout")
            for h in range(H):
                stat1 = a1[:, acol0 + h : acol0 + h + 16 * 127 + 1 : 16]
                nc.tensor.matmul(
                    po[:, 64 * h : 64 * (h + 1)], stat1,
                    xbb[:, h :: H],
                    start=True, stop=(b == NB - 1),
                )
                if b + 1 < NB:
                    stat2 = a2t[b % 2][:, h : h + 16 * 128 : 16]
                    nc.tensor.matmul(
                        po[:, 64 * h : 64 * (h + 1)], stat2,
                        xbn[:6, h :: H],
                        start=False, stop=True,
                    )
            # de-interleave h-major psum into natural channel order
            eng_copy = nc.scalar.copy if b % 2 == 0 else nc.vector.tensor_copy
            eng_copy(
                ob[:, 1024 * obhalf : 1024 * (obhalf + 1)].rearrange(
                    "p (dp h) -> p h dp", h=H
                ),
                po.rearrange("p (h dp) -> p h dp", h=H),
            )

        def run_pair(q):
            """Blocks 2q, 2q+1 -> one staging tile -> one (or two) stores."""
            ob = outs_pool.tile([128, 2 * D], F32, tag="outs")
            block(2 * q, ob, 0)
            block(2 * q + 1, ob, 1)
            r0 = 256 * q
            if q < 7:
                nc.scalar.dma_start(
                    out=AP(tensor=out_d.tensor, offset=r0 * D,
                           ap=[[D, 128], [128 * D, 2], [1, D]]),
                    in_=AP(tensor=ob[:, :].tensor, offset=0,
                           ap=[[2 * D, 128], [D, 2], [1, D]]),
                )
            else:
                nc.scalar.dma_start(
                    out=out_d[r0 : r0 + 128, :], in_=ob[:, :D]
                )
                nc.scalar.dma_start(
                    out=out_d[r0 + 128 : L, :], in_=ob[: L - r0 - 128, D:]
                )

        # ---- pipelined emission ----
        front(0)
        # img zero-fills (Pool/SWDGE): 4 slabs, once each; non-band cells
        # stay zero across reuses since every write hits the same cells
        for i in range(2):
            for s in range(2):
                nc.gpsimd.dma_start(
                    out=AP(tensor=imgp[i].tensor, offset=s * IMG_ELEMS,
                           ap=[[IMG_PITCH, 128], [1, IMG_FILL]]),
                    in_=zt[:, :],
                )
        # off-critical-path memsets: a2 tiles (cols < A2LO always zero) and
        # the et tail cols (read only for invalid outputs l >= L)
        nc.vector.memset(et[:, S - 6 :], 0.0)
        for t in a2t:
            nc.vector.memset(t, 0.0)
        front(1)
        shifts_part(0)  # et[:, 0:512); needs en cols [0, 512+6)
        t_chunks(0, 4)
        dma1p(0)
        dma1p(1)
        readp(0)
        front(2)
        shifts_part(1)
        t_chunks(4, 8)
        readp(1)
        dma1p(2)
        dma2s(0)
        dma2s(1)
        front(3)
        shifts_part(2)
        t_chunks(8, 12)
        readp(2)
        dma1p(3)
        dma2s(2)
        dma2s(3)
        t_chunks(12, 16)
        for q in range(8):
            readp(q + 3)
            dma1p(q + 4)
            dma2s(2 * q + 4)
            dma2s(2 * q + 5)
            run_pair(q)

    nc.compile()
    return nc


_CACHE = {}


def _get_program():
    if "nc" not in _CACHE:
        _CACHE["nc"] = build_program()
    return _CACHE["nc"]


def kernel(x, W, b):
    x = np.asarray(x, dtype=np.float32)
    assert x.shape == (B, S, D), x.shape

    nc = _get_program()
    consts = _host_constants(W, b)
    in_maps = []
    for core in range(B):
        in_maps.append(
            {
                "x": np.ascontiguousarray(x[core]),
                "consts": consts,
            }
        )
    res = bass_utils.run_bass_kernel_spmd(nc, in_maps, core_ids=list(range(B)))
    out = np.stack([res.results[core]["out"] for core in range(B)], axis=0)
    return out


# revision 39
# speedup vs baseline: 1.3680x; 1.3680x over previous
"""Dynamic lightweight convolution TRN2 kernel — banded-matmul design.

out[b,l,d] = (1/K) * sum_k softmax_k(x[b,l+K-1,:] @ W + bias)[k, d%H] * x[b,l+k,d]

B=8, S=2048, D=1024, K=7, H=16, L=S-K+1=2042.
Sharding: data-parallel over batch, one batch element per NeuronCore (8 cores).

Per-core plan — the conv runs on the tensor engine as banded-matrix matmuls:

  1. x is loaded by GPSIMD (SWDGE) casting DMAs: f32 HBM -> bf16 SBUF, in
     graduated batches (chunks 0,1 alone for fast pipeline start, then
     2-chunk/3-chunk batches to amortize ~1us SWDGE descriptor-gen each).
  2. Logits path: PE-transpose x chunks -> xT, logits = W^T @ xT (PE, fp32
     psum), e = exp(logits + bias) (ACT), denominators via a [112,112]
     selector matmul (PE), rinv = 1/. (DVE), en = e * rinv (DVE).  W/bias
     columns are host-permuted k -> 6-k, so row 16j+h of en is the
     normalized weight of tap k = 6-j.
  3. Shifted SBUF->SBUF DMAs build et[16j+h, s] = en[16j+h, s+j] (one DMA
     per j, batched in 3 parts gated by front(1..3)); PE-transposes give
     T[s, r] (r = 16j+h) in t_all.
  4. Band construction via a DRAM bounce (the BIR verifier rejects SBUF
     dst APs with non-partition-exact steps, so the skew must happen on
     DRAM-side strides): T chunk pairs are written to zero-filled DRAM
     images at skewed offsets SKEW*p + r and read back with row pitch
     PITCH = SKEW-16, which lands T[p, r] at band position (p, 16p + r).
     Images are paired (2 tensors x 2 slabs) so one write covers 2 chunks
     and one read covers 2 blocks — halves the per-DMA issue overhead.
  5. Conv for 128-row output block b: for each h, a banded matmul
       out[l, d'] = sum_s A1_h[s, l] * x[s, 16d'+h]
     with stationary A1_h = a1[:, h : h+2048 : 16].  The 6-row contraction
     tail (s in the next chunk) uses a2: a tiny [6, 112] load from the
     next image's left guard + a second matmul into the same psum.
  6. psum [128, 1024] (h-major) -> SBUF staging with a de-interleaving
     copy; two blocks share one staging tile and one paired store DMA.

Queue plan (per-DMA SEQ occupancy ~650ns + head-of-line waits, so queues
are balanced and ordered by data readiness):
  SP (sync):   consts, paired dma1 band writes, paired a1 band reads
  Act (scalar): et shifts, output stores (+ exp, half the ob copies)
  Pool (gpsimd): input loads, zero fills, a2 guard loads (SWDGE)
  DVE (vector): xT copies, t_all copies, recip/mul, half the ob copies
"""

import numpy as np
import ml_dtypes
from contextlib import ExitStack

import concourse.bacc as bacc
import concourse.tile as tile
from concourse import mybir
from concourse import bass_utils
from concourse.ap import AP

K = 7
H = 16
B, S, D = 8, 2048, 1024
L = S - K + 1  # 2042
C = D // 128  # 8 d-chunks
NCH = S // 128  # 16 s-chunks
NB = 16  # output blocks of 128 rows (last has 122 valid)
KH = K * H  # 112

SLOT0 = 96  # img col of (l_rel=0, h=0): band tiles are loaded from this col
ACOLS = 2048  # band-tile cols needed by the stationary views
A2LO = 1952  # a2 col of (l_rel=122, h=0); cols below are zero
A2COLS = 2064  # a2 tile width: loads land at [A2LO, A2LO+112)
IMG_PITCH = 2160  # image read pitch (elements)
IMG_SKEW = IMG_PITCH + 16  # image write pitch: +16 elems (one slot) per row
IMG_FILL = SLOT0 + ACOLS  # 2144: union of read windows per row
IMG_ELEMS = IMG_PITCH * 128  # one slab

F32 = mybir.dt.float32
BF16 = mybir.dt.bfloat16

# byte offsets (per partition) inside the packed constants blob
_OFF_BIAS = 0      # [112, 1] f32
_OFF_IDENTB = 4    # [128, 128] bf16
_OFF_SELSUM = 260  # [112, 112] bf16
_OFF_WT = 484      # [128, 8, 112] bf16
_CONST_BYTES = 2276  # 569 f32 columns


def _host_constants(W, b):
    """Pack bias/identb/selsum/W into one [128, 569] f32 blob."""
    buf = np.zeros((128, _CONST_BYTES), np.uint8)

    def put(off, arr):
        by = np.ascontiguousarray(arr).view(np.uint8).reshape(arr.shape[0], -1)
        buf[: arr.shape[0], off : off + by.shape[1]] = by

    # Permute the k-axis (k -> 6-k) of W and bias so that logits/e/en rows
    # come out in j-order (row 16j+h is the weight for tap k=6-j), matching
    # the band-image run layout r = 16j+h.
    perm = np.array([16 * (K - 1 - j) + h for j in range(K) for h in range(H)])
    put(_OFF_BIAS, np.asarray(b, np.float32)[perm].reshape(KH, 1))
    put(_OFF_IDENTB, np.eye(128).astype(ml_dtypes.bfloat16))
    hh = np.arange(KH) % H
    selsum = ((hh[:, None] == hh[None, :]) * float(K)).astype(ml_dtypes.bfloat16)
    put(_OFF_SELSUM, selsum)
    # W [D, KH] -> permuted -> [128, C, KH] chunks (d = c*128 + p)
    wt = np.asarray(W, np.float32)[:, perm].astype(ml_dtypes.bfloat16)
    wt = wt.reshape(C, 128, KH).transpose(1, 0, 2).reshape(128, C * KH)
    put(_OFF_WT, np.ascontiguousarray(wt))
    return buf.view(np.float32)


def build_program():
    nc = bacc.Bacc(
        "TRN2", target_bir_lowering=False, debug=False, enable_asserts=True
    )

    x_d = nc.dram_tensor("x", [S, D], F32, kind="ExternalInput").ap()
    consts_d = nc.dram_tensor(
        "consts", [128, _CONST_BYTES // 4], F32, kind="ExternalInput"
    ).ap()
    out_d = nc.dram_tensor("out", [L, D], F32, kind="ExternalOutput").ap()
    # three paired images, 2 slabs each (blocks 2q, 2q+1 share one tensor);
    # 3-deep rotation keeps the write(q+3)-after-read(q) WAR slack off the
    # steady-state critical path
    NIMGP = 3
    imgp = [
        nc.dram_tensor(f"imgp{i}", [2 * IMG_ELEMS], BF16, kind="Internal").ap()
        for i in range(NIMGP)
    ]

    def img_of(b):
        """(tensor ap, element offset) of block b's image slab."""
        return imgp[(b // 2) % NIMGP], (b % 2) * IMG_ELEMS

    with tile.TileContext(nc) as tc, ExitStack() as ctx:
        singles = ctx.enter_context(tc.tile_pool(name="singles", bufs=1))
        xT_pool = ctx.enter_context(tc.tile_pool(name="xT", bufs=3))
        a1_pool = ctx.enter_context(tc.tile_pool(name="a1", bufs=3))
        outs_pool = ctx.enter_context(tc.tile_pool(name="outs", bufs=3))

        p_tp = ctx.enter_context(tc.tile_pool(name="ptp", bufs=2, space="PSUM"))
        p_log = ctx.enter_context(tc.tile_pool(name="plog", bufs=1, space="PSUM"))
        p_den = ctx.enter_context(tc.tile_pool(name="pden", bufs=1, space="PSUM"))
        p_out = ctx.enter_context(tc.tile_pool(name="pout", bufs=2, space="PSUM"))

        # ---- constants: one packed DMA, tiles are views into the blob ----
        cblob = singles.tile([128, _CONST_BYTES // 4], F32)
        nc.sync.dma_start(out=cblob, in_=consts_d)
        cbytes = cblob.bitcast(mybir.dt.uint8)

        def cview(off, nbytes, dt, rows=128):
            return cbytes[:rows, off : off + nbytes].bitcast(dt)

        bias_t = cview(_OFF_BIAS, 4, F32, rows=KH)
        identb_t = cview(_OFF_IDENTB, 256, BF16)
        selsum_t = cview(_OFF_SELSUM, 224, BF16, rows=KH)
        wt = cview(_OFF_WT, 1792, BF16).rearrange("p (c n) -> p c n", c=C)

        # GPSIMD ucode warmup — dependency-free so input DMAs start at t~0
        warm = singles.tile([1, 8], BF16)
        nc.gpsimd.memset(warm, 0.0)

        # ---- persistent tensors ----
        xq = [
            singles.tile([128, 4 * D], BF16, name=f"xq{i}") for i in range(4)
        ]

        def xbv(i):
            return xq[i // 4][:, D * (i % 4) : D * (i % 4 + 1)]

        e_full = singles.tile([KH, S], BF16)
        et = singles.tile([KH, S], BF16)  # et[16j+h, s] = e[16j+h, s+j]
        den = singles.tile([16, S], BF16)  # K * sum_k e[16k+h, s]
        rall = singles.tile([128, NCH, 16], F32)  # 1/den, s-major, chunked
        t_all = singles.tile([128, NCH, KH], BF16)  # T[s, r], chunked
        zt = singles.tile([128, IMG_FILL], BF16)  # zeros for image fill
        a2t = [
            singles.tile([6, A2COLS], BF16, name=f"a2t{i}") for i in range(2)
        ]

        # ---- prologue: input DMAs first (gate the whole front pipeline) ----
        # graduated batching: chunks 0,1 alone so front(0) starts ASAP
        for lo, hi in ((0, 1), (1, 2), (2, 4), (4, 6), (6, 8), (8, 10),
                       (10, 12), (12, 16)):
            q, c0 = lo // 4, lo % 4
            assert hi - lo <= 4 and (hi - 1) // 4 == q
            nc.gpsimd.dma_start(
                out=AP(tensor=xq[q][:, :].tensor, offset=c0 * D,
                       ap=[[4 * D, 128], [D, hi - lo], [1, D]]),
                in_=AP(tensor=x_d.tensor, offset=128 * lo * D,
                       ap=[[D, 128], [128 * D, hi - lo], [1, D]]),
            )
        # zero tile for image fills (after input issue — Pool engine order)
        nc.gpsimd.memset(zt, 0.0)

        # ---- stage helpers ----
        def front(sb):
            """Transpose chunks 4sb..4sb+3, logits, exp, denom, rinv, en."""
            sl = slice(512 * sb, 512 * (sb + 1))
            xTt = xT_pool.tile([128, C, 512], BF16, tag="xT")
            for q in range(4):
                i = 4 * sb + q
                xbi = xbv(i)
                ptp = p_tp.tile([128, D], BF16, tag="ptp")
                for c in range(C):
                    nc.tensor.transpose(
                        ptp[:, 128 * c : 128 * (c + 1)],
                        xbi[:, 128 * c : 128 * (c + 1)],
                        identb_t,
                    )
                nc.vector.tensor_copy(
                    xTt[:, :, 128 * q : 128 * (q + 1)],
                    ptp.rearrange("p (c s) -> p c s", c=C),
                )
            plog = p_log.tile([KH, 512], F32, tag="plog")
            for q in range(4):
                for c in range(C):
                    nc.tensor.matmul(
                        plog[:, 128 * q : 128 * (q + 1)],
                        wt[:, c, :],
                        xTt[:, c, 128 * q : 128 * (q + 1)],
                        start=(c == 0), stop=(c == C - 1),
                    )
            nc.scalar.activation(
                e_full[:, sl], plog,
                mybir.ActivationFunctionType.Exp, bias=bias_t, scale=1.0,
            )
            # denominators only (normalization is folded into the t_all
            # copy as a reciprocal multiply — keeps the shift DMAs and the
            # band pipeline off the recip/mul critical path)
            psd = p_log.tile([16, 512], F32, tag="plog")
            nc.tensor.matmul(
                psd, selsum_t[:, :16], e_full[:, sl], start=True, stop=True
            )
            nc.vector.tensor_copy(den[:, sl], psd)

        def shifts_part(part):
            """et[16j+h, s] = e[16j+h, s+j] — one DMA per j, three batches.

            Engine copies can't start at partition 16j (BIR rule: starts must
            be 0/32/64/96) and SBUF DMA APs need partition-exact steps, so:
            plain 2-dim SBUF->SBUF DMAs, one per j-group.  Part p covers the
            et cols ready after front(p+1): [0,512), [512,1024), [1024,2048).
            Sourcing raw e (not normalized weights) means these only wait on
            exp, not on the reciprocal chain.
            """
            c0 = 512 * part
            for j in range(K):
                ln = 512 if part < 2 else 1024 - j
                nc.sync.dma_start(
                    out=AP(tensor=et[:, :].tensor, offset=16 * j * S + c0,
                           ap=[[S, 16], [1, ln]]),
                    in_=AP(tensor=e_full[:, :].tensor,
                           offset=16 * j * S + c0 + j,
                           ap=[[S, 16], [1, ln]]),
                )

        def t_chunks(lo, hi):
            for i in range(lo, hi):
                pt = p_tp.tile([128, D], BF16, tag="ptp")
                nc.tensor.transpose(
                    pt[:, :KH], et[:, 128 * i : 128 * (i + 1)],
                    identb_t[:KH, :KH],
                )
                # transposed denominators for this s-window + reciprocal
                pd = p_den.tile([128, 16], BF16, tag="pden")
                nc.tensor.transpose(
                    pd, den[:, 128 * i : 128 * (i + 1)], identb_t[:16, :16]
                )
                nc.vector.reciprocal(rall[:, i, :], pd)
                # normalized band values: T[s, 16j+h] = e_t * (1/den)[s, h]
                nc.vector.tensor_mul(
                    t_all[:, i, :], pt[:, :KH],
                    AP(tensor=rall[:, :, :].tensor, offset=16 * i,
                       ap=[[NCH * 16, 128], [0, K], [1, 16]]),
                )

        dma1_done = set()

        def dma1p(qq):
            """T chunks 2qq, 2qq+1 -> paired band image (skewed write)."""
            if qq in dma1_done or qq >= NB // 2:
                return
            dma1_done.add(qq)
            im, _ = img_of(2 * qq)
            nc.sync.dma_start(
                out=AP(tensor=im.tensor, offset=0,
                       ap=[[IMG_SKEW, 128], [IMG_ELEMS, 2], [1, KH]]),
                in_=AP(tensor=t_all[:, :, :].tensor, offset=2 * qq * KH,
                       ap=[[NCH * KH, 128], [KH, 2], [1, KH]]),
            )

        dma2s_done = set()

        def dma2s(b):
            """a2 tail for block b: the left-guard cells of block b+1's img."""
            if b in dma2s_done or b + 1 >= NB:
                return
            dma2s_done.add(b)
            im, off = img_of(b + 1)
            nc.gpsimd.dma_start(
                out=a2t[b % 2][:, A2LO : A2LO + KH],
                in_=AP(tensor=im.tensor, offset=off,
                       ap=[[IMG_PITCH, 6], [1, KH]]),
            )

        a1_tiles = {}

        def readp(qq):
            """Paired band read: blocks 2qq, 2qq+1 -> one [128, 2*ACOLS]."""
            if qq in a1_tiles or qq >= NB // 2:
                return
            im, _ = img_of(2 * qq)
            a1 = a1_pool.tile([128, 2 * ACOLS], BF16, tag="a1")
            nc.sync.dma_start(
                out=AP(tensor=a1[:, :].tensor, offset=0,
                       ap=[[2 * ACOLS, 128], [ACOLS, 2], [1, ACOLS]]),
                in_=AP(tensor=im.tensor, offset=SLOT0,
                       ap=[[IMG_PITCH, 128], [IMG_ELEMS, 2], [1, ACOLS]]),
            )
            a1_tiles[qq] = a1

        def block(b, ob, obhalf):
            """Banded conv for output rows 128b .. 128b+nl -> staging tile."""
            readp(b // 2)
            a1 = a1_tiles[b // 2]
            if b % 2 == 1:
                a1_tiles.pop(b // 2)
            acol0 = (b % 2) * ACOLS
            xbb = xbv(b)
            xbn = xbv(b + 1) if b + 1 < NB else None
            po = p_out.tile([128, D], F32, tag="pout")
            for h in range(H):
                stat1 = a1[:, acol0 + h : acol0 + h + 16 * 127 + 1 : 16]
                nc.tensor.matmul(
                    po[:, 64 * h : 64 * (h + 1)], stat1,
                    xbb[:, h :: H],
                    start=True, stop=(b == NB - 1),
                )
                if b + 1 < NB:
                    stat2 = a2t[b % 2][:, h : h + 16 * 128 : 16]
                    nc.tensor.matmul(
                        po[:, 64 * h : 64 * (h + 1)], stat2,
                        xbn[:6, h :: H],
                        start=False, stop=True,
                    )
            # de-interleave h-major psum into natural channel order
            eng_copy = nc.scalar.copy if b % 2 == 0 else nc.vector.tensor_copy
            eng_copy(
                ob[:, 1024 * obhalf : 1024 * (obhalf + 1)].rearrange(
                    "p (dp h) -> p h dp", h=H
                ),
                po.rearrange("p (h dp) -> p h dp", h=H),
            )

        def run_pair(q):
            """Blocks 2q, 2q+1 -> one staging tile -> one (or two) stores."""
            ob = outs_pool.tile([128, 2 * D], F32, tag="outs")
            block(2 * q, ob, 0)
            block(2 * q + 1, ob, 1)
            r0 = 256 * q
            if q < 7:
                nc.scalar.dma_start(
                    out=AP(tensor=out_d.tensor, offset=r0 * D,
                           ap=[[D, 128], [128 * D, 2], [1, D]]),
                    in_=AP(tensor=ob[:, :].tensor, offset=0,
                           ap=[[2 * D, 128], [D, 2], [1, D]]),
                )
            else:
                nc.scalar.dma_start(
                    out=out_d[r0 : r0 + 128, :], in_=ob[:, :D]
                )
                nc.scalar.dma_start(
                    out=out_d[r0 + 128 : L, :], in_=ob[: L - r0 - 128, D:]
                )

        # ---- pipelined emission ----
        front(0)
        # img zero-fills (Pool/SWDGE): 4 slabs, once each; non-band cells
        # stay zero across reuses since every write hits the same cells
        for i in range(2):
            for s in range(2):
                nc.gpsimd.dma_start(
                    out=AP(tensor=imgp[i].tensor, offset=s * IMG_ELEMS,
                           ap=[[IMG_PITCH, 128], [1, IMG_FILL]]),
                    in_=zt[:, :],
                )
        # off-critical-path memsets: a2 tiles (cols < A2LO always zero) and
        # the et tail cols (read only for invalid outputs l >= L)
        nc.vector.memset(et[:, S - 6 :], 0.0)
        for t in a2t:
            nc.vector.memset(t, 0.0)
        front(1)
        shifts_part(0)  # et[:, 0:512); needs e cols [0, 512+6)
        t_chunks(0, 4)
        dma1p(0)
        dma1p(1)
        readp(0)
        front(2)
        shifts_part(1)
        t_chunks(4, 8)
        dma1p(2)
        readp(1)
        dma2s(0)
        dma2s(1)
        front(3)
        shifts_part(2)
        t_chunks(8, 12)
        dma1p(3)
        readp(2)
        dma2s(2)
        dma2s(3)
        t_chunks(12, 16)
        for q in range(8):
            dma1p(q + 4)
            readp(q + 3)
            dma2s(2 * q + 4)
            dma2s(2 * q + 5)
            run_pair(q)

    nc.compile()
    return nc


_CACHE = {}


def _get_program():
    if "nc" not in _CACHE:
        _CACHE["nc"] = build_program()
    return _CACHE["nc"]


def kernel(x, W, b):
    x = np.asarray(x, dtype=np.float32)
    assert x.shape == (B, S, D), x.shape

    nc = _get_program()
    consts = _host_constants(W, b)
    in_maps = []
    for core in range(B):
        in_maps.append(
            {
                "x": np.ascontiguousarray(x[core]),
                "consts": consts,
            }
        )
    res = bass_utils.run_bass_kernel_spmd(nc, in_maps, core_ids=list(range(B)))
    out = np.stack([res.results[core]["out"] for core in range(B)], axis=0)
    return out


# revision 55
# speedup vs baseline: 1.4804x; 1.0822x over previous
"""Dynamic lightweight convolution TRN2 kernel — banded-matmul design.

out[b,l,d] = (1/K) * sum_k softmax_k(x[b,l+K-1,:] @ W + bias)[k, d%H] * x[b,l+k,d]

B=8, S=2048, D=1024, K=7, H=16, L=S-K+1=2042.
Sharding: data-parallel over batch, one batch element per NeuronCore (8 cores).

Per-core plan — the conv runs on the tensor engine as banded-matrix matmuls:

  1. x is loaded by GPSIMD (SWDGE) casting DMAs: f32 HBM -> bf16 SBUF, in
     graduated batches (chunks 0,1 alone for fast pipeline start, then
     2-chunk/3-chunk batches to amortize ~1us SWDGE descriptor-gen each).
  2. Logits path: PE-transpose x chunks -> xT, logits = W^T @ xT (PE, fp32
     psum), e = exp(logits + bias) (ACT), denominators via a [112,112]
     selector matmul (PE), rinv = 1/. (DVE), en = e * rinv (DVE).  W/bias
     columns are host-permuted k -> 6-k, so row 16j+h of en is the
     normalized weight of tap k = 6-j.
  3. Shifted SBUF->SBUF DMAs build et[16j+h, s] = en[16j+h, s+j] (one DMA
     per j, batched in 3 parts gated by front(1..3)); PE-transposes give
     T[s, r] (r = 16j+h) in t_all.
  4. Band construction via a DRAM bounce (the BIR verifier rejects SBUF
     dst APs with non-partition-exact steps, so the skew must happen on
     DRAM-side strides): T chunk pairs are written to zero-filled DRAM
     images at skewed offsets SKEW*p + r and read back with row pitch
     PITCH = SKEW-16, which lands T[p, r] at band position (p, 16p + r).
     Images are paired (2 tensors x 2 slabs) so one write covers 2 chunks
     and one read covers 2 blocks — halves the per-DMA issue overhead.
  5. Conv for 128-row output block b: for each h, a banded matmul
       out[l, d'] = sum_s A1_h[s, l] * x[s, 16d'+h]
     with stationary A1_h = a1[:, h : h+2048 : 16].  The 6-row contraction
     tail (s in the next chunk) uses a2: a tiny [6, 112] load from the
     next image's left guard + a second matmul into the same psum.
  6. psum [128, 1024] (h-major) -> SBUF staging with a de-interleaving
     copy; two blocks share one staging tile and one paired store DMA.

Queue plan (per-DMA SEQ occupancy ~650ns + head-of-line waits, so queues
are balanced and ordered by data readiness):
  SP (sync):   consts, paired dma1 band writes, paired a1 band reads
  Act (scalar): et shifts, output stores (+ exp, half the ob copies)
  Pool (gpsimd): input loads, zero fills, a2 guard loads (SWDGE)
  DVE (vector): xT copies, t_all copies, recip/mul, half the ob copies
"""

import numpy as np
import ml_dtypes
from contextlib import ExitStack

import concourse.bacc as bacc
import concourse.tile as tile
from concourse import mybir
from concourse import bass_utils
from concourse.ap import AP

K = 7
H = 16
B, S, D = 8, 2048, 1024
L = S - K + 1  # 2042
C = D // 128  # 8 d-chunks
NCH = S // 128  # 16 s-chunks
NB = 16  # output blocks of 128 rows (last has 122 valid)
KH = K * H  # 112

SLOT0 = 96  # img col of (l_rel=0, h=0): band windows are read from here
RCOLS = 1024  # band cols per 64-row sub-block window (16 per output row)
SK2 = 1152  # image write pitch: cell (s, 16j+h) lives at s*SK2 + 16j + h
P2 = SK2 - 16  # image read pitch: the -16/row slip forms the banded layout
NSLAB = 6  # rotating 128-row slabs, contiguous so reads span boundaries
IMG64_ELEMS = (NSLAB * 128 + 6) * SK2  # + 6-row wraparound guard

F32 = mybir.dt.float32
BF16 = mybir.dt.bfloat16

# byte offsets (per partition) inside the packed constants blob
_OFF_BIAS = 0      # [112, 1] f32
_OFF_IDENTB = 4    # [128, 128] bf16
_OFF_SELSUM = 260  # [112, 112] bf16
_OFF_WT = 484      # [128, 8, 112] bf16
_CONST_BYTES = 2276  # 569 f32 columns


def _host_constants(W, b):
    """Pack bias/identb/selsum/W into one [128, 569] f32 blob."""
    buf = np.zeros((128, _CONST_BYTES), np.uint8)

    def put(off, arr):
        by = np.ascontiguousarray(arr).view(np.uint8).reshape(arr.shape[0], -1)
        buf[: arr.shape[0], off : off + by.shape[1]] = by

    # Permute the k-axis (k -> 6-k) of W and bias so that logits/e/en rows
    # come out in j-order (row 16j+h is the weight for tap k=6-j), matching
    # the band-image run layout r = 16j+h.
    perm = np.array([16 * (K - 1 - j) + h for j in range(K) for h in range(H)])
    put(_OFF_BIAS, np.asarray(b, np.float32)[perm].reshape(KH, 1))
    put(_OFF_IDENTB, np.eye(128).astype(ml_dtypes.bfloat16))
    hh = np.arange(KH) % H
    selsum = ((hh[:, None] == hh[None, :]) * float(K)).astype(ml_dtypes.bfloat16)
    put(_OFF_SELSUM, selsum)
    # W [D, KH] -> permuted -> [128, C, KH] chunks (d = c*128 + p)
    wt = np.asarray(W, np.float32)[:, perm].astype(ml_dtypes.bfloat16)
    wt = wt.reshape(C, 128, KH).transpose(1, 0, 2).reshape(128, C * KH)
    put(_OFF_WT, np.ascontiguousarray(wt))
    return buf.view(np.float32)


def build_program():
    nc = bacc.Bacc(
        "TRN2", target_bir_lowering=False, debug=False, enable_asserts=True
    )

    x_d = nc.dram_tensor("x", [S, D], F32, kind="ExternalInput").ap()
    consts_d = nc.dram_tensor(
        "consts", [128, _CONST_BYTES // 4], F32, kind="ExternalInput"
    ).ap()
    out_d = nc.dram_tensor("out", [L, D], F32, kind="ExternalOutput").ap()
    # one contiguous band image: NSLAB rotating 128-row slabs + a 6-row
    # guard; chunk i writes slab i%NSLAB, and the 64-row sub-block reads
    # spill across slab boundaries (which is why slabs must be contiguous)
    img64 = nc.dram_tensor("img64", [IMG64_ELEMS], BF16, kind="Internal").ap()

    with tile.TileContext(nc) as tc, ExitStack() as ctx:
        singles = ctx.enter_context(tc.tile_pool(name="singles", bufs=1))
        xT_pool = ctx.enter_context(tc.tile_pool(name="xT", bufs=3))
        a1_pool = ctx.enter_context(tc.tile_pool(name="a1", bufs=3))
        outs_pool = ctx.enter_context(tc.tile_pool(name="outs", bufs=3))

        p_tp = ctx.enter_context(tc.tile_pool(name="ptp", bufs=2, space="PSUM"))
        p_log = ctx.enter_context(tc.tile_pool(name="plog", bufs=1, space="PSUM"))
        p_den = ctx.enter_context(tc.tile_pool(name="pden", bufs=1, space="PSUM"))
        p_out = ctx.enter_context(tc.tile_pool(name="pout", bufs=2, space="PSUM"))

        # ---- constants: identb first (gates the very first PE transpose),
        # then the rest of the packed blob ----
        cblob = singles.tile([128, _CONST_BYTES // 4], F32)
        cbytes = cblob.bitcast(mybir.dt.uint8)
        nc.sync.dma_start(
            out=cbytes[:, _OFF_IDENTB : _OFF_IDENTB + 256].bitcast(F32),
            in_=AP(tensor=consts_d.tensor, offset=_OFF_IDENTB // 4,
                   ap=[[_CONST_BYTES // 4, 128], [1, 64]]),
        )
        nc.sync.dma_start(
            out=cbytes[:, _OFF_SELSUM:].bitcast(F32),
            in_=AP(tensor=consts_d.tensor, offset=_OFF_SELSUM // 4,
                   ap=[[_CONST_BYTES // 4, 128],
                       [1, (_CONST_BYTES - _OFF_SELSUM) // 4]]),
        )
        nc.sync.dma_start(
            out=cbytes[:, :_OFF_IDENTB].bitcast(F32),
            in_=AP(tensor=consts_d.tensor, offset=0,
                   ap=[[_CONST_BYTES // 4, 128], [1, _OFF_IDENTB // 4]]),
        )

        def cview(off, nbytes, dt, rows=128):
            return cbytes[:rows, off : off + nbytes].bitcast(dt)

        bias_t = cview(_OFF_BIAS, 4, F32, rows=KH)
        identb_t = cview(_OFF_IDENTB, 256, BF16)
        selsum_t = cview(_OFF_SELSUM, 224, BF16, rows=KH)
        wt = cview(_OFF_WT, 1792, BF16).rearrange("p (c n) -> p c n", c=C)

        # GPSIMD ucode warmup — dependency-free so input DMAs start at t~0
        warm = singles.tile([1, 8], BF16)
        nc.gpsimd.memset(warm, 0.0)

        # ---- persistent tensors ----
        xq = [
            singles.tile([128, 4 * D], BF16, name=f"xq{i}") for i in range(4)
        ]

        def xbv(i):
            return xq[i // 4][:, D * (i % 4) : D * (i % 4 + 1)]

        e_full = singles.tile([KH, S], BF16)
        et = singles.tile([KH, S], BF16)  # et[16j+h, s] = e[16j+h, s+j]
        den = singles.tile([16, S], BF16)  # K * sum_k e[16k+h, s]
        rall = singles.tile([128, NCH, 16], F32)  # 1/den, s-major, chunked
        t_all = singles.tile([128, NCH, KH], BF16)  # T[s, r], chunked
        zt = singles.tile([128, SK2], BF16)  # zeros for image fill

        # ---- prologue ----
        # zero tile on DVE: fits in DVE's idle window before the first xT
        # copy, and keeps Pool free for input descriptor generation
        nc.vector.memset(zt, 0.0)
        # input DMAs gate the whole front pipeline;
        # graduated batching: chunks 0,1 alone so front(0) starts ASAP
        for lo, hi in ((0, 1), (1, 2), (2, 4), (4, 6), (6, 8), (8, 10),
                       (10, 12), (12, 16)):
            q, c0 = lo // 4, lo % 4
            assert hi - lo <= 4 and (hi - 1) // 4 == q
            nc.gpsimd.dma_start(
                out=AP(tensor=xq[q][:, :].tensor, offset=c0 * D,
                       ap=[[4 * D, 128], [D, hi - lo], [1, D]]),
                in_=AP(tensor=x_d.tensor, offset=128 * lo * D,
                       ap=[[D, 128], [128 * D, hi - lo], [1, D]]),
            )

        # ---- stage helpers ----
        def front(sb):
            """Transpose chunks 4sb..4sb+3, logits, exp, denom, rinv, en."""
            sl = slice(512 * sb, 512 * (sb + 1))
            xTt = xT_pool.tile([128, C, 512], BF16, tag="xT")
            for q in range(4):
                i = 4 * sb + q
                xbi = xbv(i)
                ptp = p_tp.tile([128, D], BF16, tag="ptp")
                for c in range(C):
                    nc.tensor.transpose(
                        ptp[:, 128 * c : 128 * (c + 1)],
                        xbi[:, 128 * c : 128 * (c + 1)],
                        identb_t,
                    )
                cp = nc.scalar.copy if q == 3 else nc.vector.tensor_copy
                cp(
                    xTt[:, :, 128 * q : 128 * (q + 1)],
                    ptp.rearrange("p (c s) -> p c s", c=C),
                )
            plog = p_log.tile([KH, 512], F32, tag="plog")
            for q in range(4):
                for c in range(C):
                    nc.tensor.matmul(
                        plog[:, 128 * q : 128 * (q + 1)],
                        wt[:, c, :],
                        xTt[:, c, 128 * q : 128 * (q + 1)],
                        start=(c == 0), stop=(c == C - 1),
                    )
            nc.scalar.activation(
                e_full[:, sl], plog,
                mybir.ActivationFunctionType.Exp, bias=bias_t, scale=1.0,
            )
            # denominators only (normalization is folded into the t_all
            # copy as a reciprocal multiply — keeps the shift DMAs and the
            # band pipeline off the recip/mul critical path)
            psd = p_log.tile([16, 512], F32, tag="plog")
            nc.tensor.matmul(
                psd, selsum_t[:, :16], e_full[:, sl], start=True, stop=True
            )
            nc.vector.tensor_copy(den[:, sl], psd)

        def shifts_part(part):
            """et[16j+h, s] = e[16j+h, s+j] — one DMA per j, three batches.

            Engine copies can't start at partition 16j (BIR rule: starts must
            be 0/32/64/96) and SBUF DMA APs need partition-exact steps, so:
            plain 2-dim SBUF->SBUF DMAs, one per j-group.  Part p covers the
            et cols ready after front(p+1): [0,512), [512,1024), [1024,2048).
            Sourcing raw e (not normalized weights) means these only wait on
            exp, not on the reciprocal chain.
            """
            c0 = 512 * part
            for j in range(K):
                ln = 512 if part < 2 else 1024 - j
                nc.sync.dma_start(
                    out=AP(tensor=et[:, :].tensor, offset=16 * j * S + c0,
                           ap=[[S, 16], [1, ln]]),
                    in_=AP(tensor=e_full[:, :].tensor,
                           offset=16 * j * S + c0 + j,
                           ap=[[S, 16], [1, ln]]),
                )

        def t_chunks(lo, hi):
            for i in range(lo, hi):
                pt = p_tp.tile([128, D], BF16, tag="ptp")
                nc.tensor.transpose(
                    pt[:, :KH], et[:, 128 * i : 128 * (i + 1)],
                    identb_t[:KH, :KH],
                )
                # transposed denominators for this s-window + reciprocal
                pd = p_den.tile([128, 16], BF16, tag="pden")
                nc.tensor.transpose(
                    pd, den[:, 128 * i : 128 * (i + 1)], identb_t[:16, :16]
                )
                nc.vector.reciprocal(rall[:, i, :], pd)
                # normalized band values: T[s, 16j+h] = e_t * (1/den)[s, h]
                nc.vector.tensor_mul(
                    t_all[:, i, :], pt[:, :KH],
                    AP(tensor=rall[:, :, :].tensor, offset=16 * i,
                       ap=[[NCH * 16, 128], [0, K], [1, 16]]),
                )

        dma1_done = set()

        def dma1p(u):
            """T chunks 2u, 2u+1 -> adjacent band-image slabs (skewed)."""
            if u in dma1_done or u >= NB // 2:
                return
            dma1_done.add(u)
            nc.sync.dma_start(
                out=AP(tensor=img64.tensor,
                       offset=(2 * u % NSLAB) * 128 * SK2,
                       ap=[[SK2, 128], [128 * SK2, 2], [1, KH]]),
                in_=AP(tensor=t_all[:, :, :].tensor, offset=2 * u * KH,
                       ap=[[NCH * KH, 128], [KH, 2], [1, KH]]),
            )

        def dup_guard(i):
            """Duplicate chunk i's first 6 T-rows into the wraparound guard.

            Chunk i-1's sub-block-1 read spills 6 rows past slab NSLAB-1;
            when i % NSLAB == 0 those cells physically live in the guard
            instead of slab 0, so write them twice (idempotent values)."""
            nc.sync.dma_start(
                out=AP(tensor=img64.tensor, offset=NSLAB * 128 * SK2,
                       ap=[[SK2, 6], [1, KH]]),
                in_=AP(tensor=t_all[:, :, :].tensor, offset=i * KH,
                       ap=[[NCH * KH, 6], [1, KH]]),
            )

        a1_tiles = {}

        def readp(u):
            """Band reads for chunks 2u, 2u+1 -> one [128, 2*2048] tile.

            PE matmuls need stationary and moving operands on the same base
            partitions, so the three window pieces land where the x rows
            live: sub-0 window at partitions 0:70, sub-1 main at 64:128,
            sub-1 tail (chunk b+1 rows, crossing the slab boundary — slabs
            contiguous) at 0:6.  Chunk-paired 3-dim DMAs: 3 issues/pair.
            """
            if u in a1_tiles or u >= NB // 2:
                return
            a1 = a1_pool.tile([128, 2 * 2048], BF16, tag="a1")
            beta0 = (2 * u % NSLAB) * 128 * SK2 + SLOT0
            beta1 = beta0 + 64 * SK2
            nc.scalar.dma_start(
                out=AP(tensor=a1[:, :].tensor, offset=0,
                       ap=[[4096, 70], [2048, 2], [1, RCOLS]]),
                in_=AP(tensor=img64.tensor, offset=beta0,
                       ap=[[P2, 70], [128 * SK2, 2], [1, RCOLS]]),
            )
            nc.scalar.dma_start(
                out=AP(tensor=a1[:, :].tensor, offset=64 * 4096 + RCOLS,
                       ap=[[4096, 64], [2048, 2], [1, RCOLS]]),
                in_=AP(tensor=img64.tensor, offset=beta1,
                       ap=[[P2, 64], [128 * SK2, 2], [1, RCOLS]]),
            )
            nc.scalar.dma_start(
                out=AP(tensor=a1[:, :].tensor, offset=RCOLS,
                       ap=[[4096, 6], [2048, 2], [1, RCOLS]]),
                in_=AP(tensor=img64.tensor, offset=beta1 + 64 * P2,
                       ap=[[P2, 6], [128 * SK2, 2], [1, RCOLS]]),
            )
            a1_tiles[u] = a1

        def block(b, ob, obhalf):
            """Banded conv for output rows 128b .. 128b+nl -> staging tile.

            Two 64-row sub-blocks per chunk: sub 0's 70-row s-window sits
            inside chunk b (single matmul per h); sub 1 splits into a 64-row
            main (chunk b rows 64:128) plus a 6-row tail from chunk b+1.
            Matmul partition bases 0/64 are legal (0/32/64/96 rule)."""
            readp(b // 2)
            a1 = a1_tiles[b // 2]
            if b % 2 == 1:
                a1_tiles.pop(b // 2)
            cb = (b % 2) * 2048
            xbb = xbv(b)
            xbn = xbv(b + 1) if b + 1 < NB else None
            po = p_out.tile([128, D], F32, tag="pout")
            for h in range(H):
                cl = slice(64 * h, 64 * (h + 1))
                nc.tensor.matmul(
                    po[:64, cl], a1[0:70, cb + h : cb + h + 16 * 63 + 1 : 16],
                    xbb[0:70, h :: H],
                    start=True, stop=True,
                )
                c1 = cb + RCOLS + h
                nc.tensor.matmul(
                    po[64:, cl], a1[64:128, c1 : c1 + 16 * 63 + 1 : 16],
                    xbb[64:128, h :: H],
                    start=True, stop=(b == NB - 1),
                )
                if b + 1 < NB:
                    nc.tensor.matmul(
                        po[64:, cl], a1[0:6, c1 : c1 + 16 * 63 + 1 : 16],
                        xbn[0:6, h :: H],
                        start=False, stop=True,
                    )
            # de-interleave h-major psum into natural channel order
            eng_copy = nc.scalar.copy if b % 2 == 0 else nc.vector.tensor_copy
            eng_copy(
                ob[:, 1024 * obhalf : 1024 * (obhalf + 1)].rearrange(
                    "p (dp h) -> p h dp", h=H
                ),
                po.rearrange("p (h dp) -> p h dp", h=H),
            )

        def run_pair(q):
            """Blocks 2q, 2q+1 -> one staging tile -> one (or two) stores."""
            ob = outs_pool.tile([128, 2 * D], F32, tag="outs")
            block(2 * q, ob, 0)
            block(2 * q + 1, ob, 1)
            r0 = 256 * q
            if q < 7:
                nc.scalar.dma_start(
                    out=AP(tensor=out_d.tensor, offset=r0 * D,
                           ap=[[D, 128], [128 * D, 2], [1, D]]),
                    in_=AP(tensor=ob[:, :].tensor, offset=0,
                           ap=[[2 * D, 128], [D, 2], [1, D]]),
                )
            else:
                nc.scalar.dma_start(
                    out=out_d[r0 : r0 + 128, :], in_=ob[:, :D]
                )
                nc.scalar.dma_start(
                    out=out_d[r0 + 128 : L, :], in_=ob[: L - r0 - 128, D:]
                )

        # ---- pipelined emission ----
        front(0)
        # img zero-fills (Pool/SWDGE): 4 slabs, once each; non-band cells
        # stay zero across reuses since every write hits the same cells
        for s in range(NSLAB):
            nc.gpsimd.dma_start(
                out=AP(tensor=img64.tensor, offset=s * 128 * SK2,
                       ap=[[SK2, 128], [1, SK2]]),
                in_=zt[:, :],
            )
        nc.gpsimd.dma_start(
            out=AP(tensor=img64.tensor, offset=NSLAB * 128 * SK2,
                   ap=[[SK2, 6], [1, SK2]]),
            in_=zt[:6, :],
        )
        # et tail cols: read only for invalid outputs l >= L; keep finite
        nc.vector.memset(et[:, S - 6 :], 0.0)
        front(1)
        shifts_part(0)  # et[:, 0:512); needs e cols [0, 512+6)
        t_chunks(0, 4)
        dma1p(0)
        dma1p(1)
        readp(0)
        front(2)
        shifts_part(1)
        t_chunks(4, 8)
        dma1p(2)
        readp(1)
        dma1p(3)
        dup_guard(6)
        readp(2)
        front(3)
        shifts_part(2)
        t_chunks(8, 12)
        dma1p(4)
        readp(3)
        dma1p(5)
        readp(4)
        t_chunks(12, 16)
        dma1p(6)
        dup_guard(12)
        readp(5)
        for q in range(8):
            dma1p(q + 7)
            readp(q + 6)
            run_pair(q)

    nc.compile()
    return nc


_CACHE = {}


def _get_program():
    if "nc" not in _CACHE:
        _CACHE["nc"] = build_program()
    return _CACHE["nc"]


def kernel(x, W, b):
    x = np.asarray(x, dtype=np.float32)
    assert x.shape == (B, S, D), x.shape

    nc = _get_program()
    consts = _host_constants(W, b)
    in_maps = []
    for core in range(B):
        in_maps.append(
            {
                "x": np.ascontiguousarray(x[core]),
                "consts": consts,
            }
        )
    res = bass_utils.run_bass_kernel_spmd(nc, in_maps, core_ids=list(range(B)))
    out = np.stack([res.results[core]["out"] for core in range(B)], axis=0)
    return out


# revision 71
# speedup vs baseline: 1.5481x; 1.0457x over previous
"""Dynamic lightweight convolution TRN2 kernel — banded-matmul design.

out[b,l,d] = (1/K) * sum_k softmax_k(x[b,l+K-1,:] @ W + bias)[k, d%H] * x[b,l+k,d]

B=8, S=2048, D=1024, K=7, H=16, L=S-K+1=2042.
Sharding: data-parallel over batch, one batch element per NeuronCore (8 cores).

Per-core plan — the conv runs on the tensor engine as banded-matrix matmuls:

  1. x is loaded by GPSIMD (SWDGE) casting DMAs: f32 HBM -> bf16 SBUF, in
     graduated batches (chunks 0,1 alone for fast pipeline start, then
     2-chunk/3-chunk batches to amortize ~1us SWDGE descriptor-gen each).
  2. Logits path: PE-transpose x chunks -> xT, logits = W^T @ xT (PE, fp32
     psum), e = exp(logits + bias) (ACT), denominators via a [112,112]
     selector matmul (PE), rinv = 1/. (DVE), en = e * rinv (DVE).  W/bias
     columns are host-permuted k -> 6-k, so row 16j+h of en is the
     normalized weight of tap k = 6-j.
  3. Shifted SBUF->SBUF DMAs build et[16j+h, s] = en[16j+h, s+j] (one DMA
     per j, batched in 3 parts gated by front(1..3)); PE-transposes give
     T[s, r] (r = 16j+h) in t_all.
  4. Band construction via a DRAM bounce (the BIR verifier rejects SBUF
     dst APs with non-partition-exact steps, so the skew must happen on
     DRAM-side strides): T chunk pairs are written to zero-filled DRAM
     images at skewed offsets SKEW*p + r and read back with row pitch
     PITCH = SKEW-16, which lands T[p, r] at band position (p, 16p + r).
     Images are paired (2 tensors x 2 slabs) so one write covers 2 chunks
     and one read covers 2 blocks — halves the per-DMA issue overhead.
  5. Conv for 128-row output block b: for each h, a banded matmul
       out[l, d'] = sum_s A1_h[s, l] * x[s, 16d'+h]
     with stationary A1_h = a1[:, h : h+2048 : 16].  The 6-row contraction
     tail (s in the next chunk) uses a2: a tiny [6, 112] load from the
     next image's left guard + a second matmul into the same psum.
  6. psum [128, 1024] (h-major) -> SBUF staging with a de-interleaving
     copy; two blocks share one staging tile and one paired store DMA.

Queue plan (per-DMA SEQ occupancy ~650ns + head-of-line waits, so queues
are balanced and ordered by data readiness):
  SP (sync):   consts, paired dma1 band writes, paired a1 band reads
  Act (scalar): et shifts, output stores (+ exp, half the ob copies)
  Pool (gpsimd): input loads, zero fills, a2 guard loads (SWDGE)
  DVE (vector): xT copies, t_all copies, recip/mul, half the ob copies
"""

import numpy as np
import ml_dtypes
from contextlib import ExitStack

import concourse.bacc as bacc
import concourse.tile as tile
from concourse import mybir
from concourse import bass_utils
from concourse.ap import AP

K = 7
H = 16
B, S, D = 8, 2048, 1024
L = S - K + 1  # 2042
C = D // 128  # 8 d-chunks
NCH = S // 128  # 16 s-chunks
NB = 16  # output blocks of 128 rows (last has 122 valid)
KH = K * H  # 112

SLOT0 = 96  # img col of (l_rel=0, h=0): band windows are read from here
RCOLS = 1024  # band cols per 64-row sub-block window (16 per output row)
SK2 = 1152  # image write pitch: cell (s, 16j+h) lives at s*SK2 + 16j + h
P2 = SK2 - 16  # image read pitch: the -16/row slip forms the banded layout
NSLAB = 6  # rotating 128-row slabs, contiguous so reads span boundaries
IMG64_ELEMS = (NSLAB * 128 + 6) * SK2  # + 6-row wraparound guard

F32 = mybir.dt.float32
BF16 = mybir.dt.bfloat16

# byte offsets (per partition) inside the packed constants blob
_OFF_BIAS = 0      # [112, 1] f32
_OFF_IDENTB = 4    # [128, 128] bf16
_OFF_SELSUM = 260  # [112, 112] bf16
_OFF_WT = 484      # [128, 8, 112] bf16
_CONST_BYTES = 2276  # 569 f32 columns


def _host_constants(W, b):
    """Pack bias/identb/selsum/W into one [128, 569] f32 blob."""
    buf = np.zeros((128, _CONST_BYTES), np.uint8)

    def put(off, arr):
        by = np.ascontiguousarray(arr).view(np.uint8).reshape(arr.shape[0], -1)
        buf[: arr.shape[0], off : off + by.shape[1]] = by

    # Permute the k-axis (k -> 6-k) of W and bias so that logits/e/en rows
    # come out in j-order (row 16j+h is the weight for tap k=6-j), matching
    # the band-image run layout r = 16j+h.
    perm = np.array([16 * (K - 1 - j) + h for j in range(K) for h in range(H)])
    put(_OFF_BIAS, np.asarray(b, np.float32)[perm].reshape(KH, 1))
    put(_OFF_IDENTB, np.eye(128).astype(ml_dtypes.bfloat16))
    hh = np.arange(KH) % H
    selsum = ((hh[:, None] == hh[None, :]) * float(K)).astype(ml_dtypes.bfloat16)
    put(_OFF_SELSUM, selsum)
    # W [D, KH] -> permuted -> [128, C, KH] chunks (d = c*128 + p)
    wt = np.asarray(W, np.float32)[:, perm].astype(ml_dtypes.bfloat16)
    wt = wt.reshape(C, 128, KH).transpose(1, 0, 2).reshape(128, C * KH)
    put(_OFF_WT, np.ascontiguousarray(wt))
    return buf.view(np.float32)


def build_program():
    nc = bacc.Bacc(
        "TRN2", target_bir_lowering=False, debug=False, enable_asserts=True
    )

    x_d = nc.dram_tensor("x", [S, D], F32, kind="ExternalInput").ap()
    consts_d = nc.dram_tensor(
        "consts", [128, _CONST_BYTES // 4], F32, kind="ExternalInput"
    ).ap()
    out_d = nc.dram_tensor("out", [L, D], F32, kind="ExternalOutput").ap()
    # one contiguous band image: NSLAB rotating 128-row slabs + a 6-row
    # guard; chunk i writes slab i%NSLAB, and the 64-row sub-block reads
    # spill across slab boundaries (which is why slabs must be contiguous)
    img64 = nc.dram_tensor("img64", [IMG64_ELEMS], BF16, kind="Internal").ap()

    with tile.TileContext(nc) as tc, ExitStack() as ctx:
        singles = ctx.enter_context(tc.tile_pool(name="singles", bufs=1))
        xT_pool = ctx.enter_context(tc.tile_pool(name="xT", bufs=3))
        a1_pool = ctx.enter_context(tc.tile_pool(name="a1", bufs=3))
        outs_pool = ctx.enter_context(tc.tile_pool(name="outs", bufs=3))

        p_tp = ctx.enter_context(tc.tile_pool(name="ptp", bufs=2, space="PSUM"))
        p_log = ctx.enter_context(tc.tile_pool(name="plog", bufs=1, space="PSUM"))
        p_den = ctx.enter_context(tc.tile_pool(name="pden", bufs=1, space="PSUM"))
        p_out = ctx.enter_context(tc.tile_pool(name="pout", bufs=2, space="PSUM"))

        # ---- constants: identb first (gates the very first PE transpose),
        # then the rest of the packed blob ----
        cblob = singles.tile([128, _CONST_BYTES // 4], F32)
        cbytes = cblob.bitcast(mybir.dt.uint8)
        nc.sync.dma_start(
            out=cbytes[:, _OFF_IDENTB : _OFF_IDENTB + 256].bitcast(F32),
            in_=AP(tensor=consts_d.tensor, offset=_OFF_IDENTB // 4,
                   ap=[[_CONST_BYTES // 4, 128], [1, 64]]),
        )
        nc.sync.dma_start(
            out=cbytes[:, _OFF_SELSUM:].bitcast(F32),
            in_=AP(tensor=consts_d.tensor, offset=_OFF_SELSUM // 4,
                   ap=[[_CONST_BYTES // 4, 128],
                       [1, (_CONST_BYTES - _OFF_SELSUM) // 4]]),
        )
        nc.sync.dma_start(
            out=cbytes[:, :_OFF_IDENTB].bitcast(F32),
            in_=AP(tensor=consts_d.tensor, offset=0,
                   ap=[[_CONST_BYTES // 4, 128], [1, _OFF_IDENTB // 4]]),
        )

        def cview(off, nbytes, dt, rows=128):
            return cbytes[:rows, off : off + nbytes].bitcast(dt)

        bias_t = cview(_OFF_BIAS, 4, F32, rows=KH)
        identb_t = cview(_OFF_IDENTB, 256, BF16)
        selsum_t = cview(_OFF_SELSUM, 224, BF16, rows=KH)
        wt = cview(_OFF_WT, 1792, BF16).rearrange("p (c n) -> p c n", c=C)

        # GPSIMD ucode warmup — dependency-free so input DMAs start at t~0
        warm = singles.tile([1, 8], BF16)
        nc.gpsimd.memset(warm, 0.0)

        # PE p-state warmup: the cost model runs matmuls at half clock until
        # the engine has been continuously busy for 3us.  Burn the ramp on
        # dummy matmuls while the first input chunk is still in flight.
        junk = singles.tile([128, 128], BF16)
        nc.vector.memset(junk, 0.0)
        pwarm = p_out.tile([128, D], F32, tag="pout")
        for _ in range(44):
            nc.tensor.matmul(pwarm[:, :128], junk, junk, start=True, stop=True)

        # ---- persistent tensors ----
        xq = [
            singles.tile([128, 4 * D], BF16, name=f"xq{i}") for i in range(4)
        ]

        def xbv(i):
            return xq[i // 4][:, D * (i % 4) : D * (i % 4 + 1)]

        e_full = singles.tile([KH, S], BF16)
        et = singles.tile([KH, S], BF16)  # et[16j+h, s] = e[16j+h, s+j]
        den = singles.tile([16, S + 8], BF16)  # K * sum_k e[16k+h, s]
        rallo = singles.tile([128, NB, 16], F32)  # 1/den[h, l+6], l-major
        t_all = singles.tile([128, NCH, KH], BF16)  # T[s, r], chunked
        zt = singles.tile([128, SK2], BF16)  # zeros for image fill

        # ---- prologue ----
        # zero tile on DVE: fits in DVE's idle window before the first xT
        # copy, and keeps Pool free for input descriptor generation
        nc.vector.memset(zt, 0.0)
        # input DMAs gate the whole front pipeline;
        # graduated batching: chunks 0,1 alone so front(0) starts ASAP
        for lo, hi in ((0, 1), (1, 2), (2, 4), (4, 6), (6, 8), (8, 10),
                       (10, 12), (12, 16)):
            q, c0 = lo // 4, lo % 4
            assert hi - lo <= 4 and (hi - 1) // 4 == q
            nc.gpsimd.dma_start(
                out=AP(tensor=xq[q][:, :].tensor, offset=c0 * D,
                       ap=[[4 * D, 128], [D, hi - lo], [1, D]]),
                in_=AP(tensor=x_d.tensor, offset=128 * lo * D,
                       ap=[[D, 128], [128 * D, hi - lo], [1, D]]),
            )

        # ---- stage helpers ----
        def front(sb):
            """Transpose chunks 4sb..4sb+3, logits, exp, denom, rinv, en."""
            sl = slice(512 * sb, 512 * (sb + 1))
            xTt = xT_pool.tile([128, C, 512], BF16, tag="xT")
            for q in range(4):
                i = 4 * sb + q
                xbi = xbv(i)
                ptp = p_tp.tile([128, D], BF16, tag="ptp")
                for c in range(C):
                    nc.tensor.transpose(
                        ptp[:, 128 * c : 128 * (c + 1)],
                        xbi[:, 128 * c : 128 * (c + 1)],
                        identb_t,
                    )
                cp = nc.vector.tensor_copy if True else nc.scalar.copy
                cp(
                    xTt[:, :, 128 * q : 128 * (q + 1)],
                    ptp.rearrange("p (c s) -> p c s", c=C),
                )
            plog = p_log.tile([KH, 512], F32, tag="plog")
            for q in range(4):
                for c in range(C):
                    nc.tensor.matmul(
                        plog[:, 128 * q : 128 * (q + 1)],
                        wt[:, c, :],
                        xTt[:, c, 128 * q : 128 * (q + 1)],
                        start=(c == 0), stop=(c == C - 1),
                    )
            nc.scalar.activation(
                e_full[:, sl], plog,
                mybir.ActivationFunctionType.Exp, bias=bias_t, scale=1.0,
            )
            # denominators only (normalization is folded into the t_all
            # copy as a reciprocal multiply — keeps the shift DMAs and the
            # band pipeline off the recip/mul critical path)
            psd = p_log.tile([16, 512], F32, tag="plog")
            nc.tensor.matmul(
                psd, selsum_t[:, :16], e_full[:, sl], start=True, stop=True
            )
            nc.vector.tensor_copy(den[:, sl], psd)

        def shifts_part(part):
            """et[16j+h, s] = e[16j+h, s+j] — one DMA per j, three batches.

            Engine copies can't start at partition 16j (BIR rule: starts must
            be 0/32/64/96) and SBUF DMA APs need partition-exact steps, so:
            plain 2-dim SBUF->SBUF DMAs, one per j-group.  Part p covers the
            et cols ready after front(p+1): [0,512), [512,1024), [1024,2048).
            Sourcing raw e (not normalized weights) means these only wait on
            exp, not on the reciprocal chain.
            """
            c0 = 512 * part
            for j in range(K):
                ln = 512 if part < 2 else 1024 - j
                nc.scalar.dma_start(
                    out=AP(tensor=et[:, :].tensor, offset=16 * j * S + c0,
                           ap=[[S, 16], [1, ln]]),
                    in_=AP(tensor=e_full[:, :].tensor,
                           offset=16 * j * S + c0 + j,
                           ap=[[S, 16], [1, ln]]),
                )

        def t_chunks(lo, hi):
            for i in range(lo, hi):
                pt = p_tp.tile([128, D], BF16, tag="ptp")
                nc.tensor.transpose(
                    pt[:, :KH], et[:, 128 * i : 128 * (i + 1)],
                    identb_t[:KH, :KH],
                )
                nc.vector.tensor_copy(t_all[:, i, :], pt[:, :KH])

        def rall_block(b):
            """1/den for block b's output rows: rallo[l_rel, b, h] =
            1/den[h, 128b + l_rel + 6].  The band carries raw exp values;
            the softmax denominator (indexed at l+6) is applied to the
            output, folded into the de-interleave as a multiply."""
            pd = p_den.tile([128, 16], BF16, tag="pden")
            nc.tensor.transpose(
                pd, den[:, 128 * b + 6 : 128 * b + 134], identb_t[:16, :16]
            )
            nc.vector.reciprocal(rallo[:, b, :], pd)

        dma1_done = set()

        def dma1p(u):
            """T chunks 2u, 2u+1 -> adjacent band-image slabs (skewed)."""
            if u in dma1_done or u >= NB // 2:
                return
            dma1_done.add(u)
            nc.sync.dma_start(
                out=AP(tensor=img64.tensor,
                       offset=(2 * u % NSLAB) * 128 * SK2,
                       ap=[[SK2, 128], [128 * SK2, 2], [1, KH]]),
                in_=AP(tensor=t_all[:, :, :].tensor, offset=2 * u * KH,
                       ap=[[NCH * KH, 128], [KH, 2], [1, KH]]),
            )

        def dup_guard(i):
            """Duplicate chunk i's first 6 T-rows into the wraparound guard.

            Chunk i-1's sub-block-1 read spills 6 rows past slab NSLAB-1;
            when i % NSLAB == 0 those cells physically live in the guard
            instead of slab 0, so write them twice (idempotent values)."""
            nc.sync.dma_start(
                out=AP(tensor=img64.tensor, offset=NSLAB * 128 * SK2,
                       ap=[[SK2, 6], [1, KH]]),
                in_=AP(tensor=t_all[:, :, :].tensor, offset=i * KH,
                       ap=[[NCH * KH, 6], [1, KH]]),
            )

        a1_tiles = {}

        def readp(u):
            """Band reads for chunks 2u, 2u+1 -> one [128, 2*2048] tile.

            PE matmuls need stationary and moving operands on the same base
            partitions, so the three window pieces land where the x rows
            live: sub-0 window at partitions 0:70, sub-1 main at 64:128,
            sub-1 tail (chunk b+1 rows, crossing the slab boundary — slabs
            contiguous) at 0:6.  Chunk-paired 3-dim DMAs: 3 issues/pair.
            """
            if u in a1_tiles or u >= NB // 2:
                return
            a1 = a1_pool.tile([128, 2 * 2048], BF16, tag="a1")
            beta0 = (2 * u % NSLAB) * 128 * SK2 + SLOT0
            beta1 = beta0 + 64 * SK2
            nc.sync.dma_start(
                out=AP(tensor=a1[:, :].tensor, offset=0,
                       ap=[[4096, 70], [2048, 2], [1, RCOLS]]),
                in_=AP(tensor=img64.tensor, offset=beta0,
                       ap=[[P2, 70], [128 * SK2, 2], [1, RCOLS]]),
            )
            nc.sync.dma_start(
                out=AP(tensor=a1[:, :].tensor, offset=64 * 4096 + RCOLS,
                       ap=[[4096, 64], [2048, 2], [1, RCOLS]]),
                in_=AP(tensor=img64.tensor, offset=beta1,
                       ap=[[P2, 64], [128 * SK2, 2], [1, RCOLS]]),
            )
            nc.sync.dma_start(
                out=AP(tensor=a1[:, :].tensor, offset=RCOLS,
                       ap=[[4096, 6], [2048, 2], [1, RCOLS]]),
                in_=AP(tensor=img64.tensor, offset=beta1 + 64 * P2,
                       ap=[[P2, 6], [128 * SK2, 2], [1, RCOLS]]),
            )
            a1_tiles[u] = a1

        def block(b, ob, obhalf):
            """Banded conv for output rows 128b .. 128b+nl -> staging tile.

            Two 64-row sub-blocks per chunk: sub 0's 70-row s-window sits
            inside chunk b (single matmul per h); sub 1 splits into a 64-row
            main (chunk b rows 64:128) plus a 6-row tail from chunk b+1.
            Matmul partition bases 0/64 are legal (0/32/64/96 rule)."""
            readp(b // 2)
            a1 = a1_tiles[b // 2]
            if b % 2 == 1:
                a1_tiles.pop(b // 2)
            rall_block(b)
            cb = (b % 2) * 2048
            xbb = xbv(b)
            xbn = xbv(b + 1) if b + 1 < NB else None
            po = p_out.tile([128, D], F32, tag="pout")
            for h in range(H):
                cl = slice(64 * h, 64 * (h + 1))
                nc.tensor.matmul(
                    po[:64, cl], a1[0:70, cb + h : cb + h + 16 * 63 + 1 : 16],
                    xbb[0:70, h :: H],
                    start=True, stop=True,
                )
                c1 = cb + RCOLS + h
                nc.tensor.matmul(
                    po[64:, cl], a1[64:128, c1 : c1 + 16 * 63 + 1 : 16],
                    xbb[64:128, h :: H],
                    start=True, stop=(b == NB - 1),
                )
                if b + 1 < NB:
                    nc.tensor.matmul(
                        po[64:, cl], a1[0:6, c1 : c1 + 16 * 63 + 1 : 16],
                        xbn[0:6, h :: H],
                        start=False, stop=True,
                    )
            # de-interleave h-major psum into natural channel order, folding
            # in the softmax denominator (broadcast over the 64 d' lanes)
            nc.vector.tensor_mul(
                ob[:, 1024 * obhalf : 1024 * (obhalf + 1)].rearrange(
                    "p (dp h) -> p h dp", h=H
                ),
                po.rearrange("p (h dp) -> p h dp", h=H),
                AP(tensor=rallo[:, :, :].tensor, offset=16 * b,
                   ap=[[NB * 16, 128], [1, 16], [0, 64]]),
            )

        def run_pair(q):
            """Blocks 2q, 2q+1 -> one staging tile -> one (or two) stores."""
            ob = outs_pool.tile([128, 2 * D], F32, tag="outs")
            block(2 * q, ob, 0)
            block(2 * q + 1, ob, 1)
            r0 = 256 * q
            if q < 7:
                nc.scalar.dma_start(
                    out=AP(tensor=out_d.tensor, offset=r0 * D,
                           ap=[[D, 128], [128 * D, 2], [1, D]]),
                    in_=AP(tensor=ob[:, :].tensor, offset=0,
                           ap=[[2 * D, 128], [D, 2], [1, D]]),
                )
            else:
                nc.scalar.dma_start(
                    out=out_d[r0 : r0 + 128, :], in_=ob[:, :D]
                )
                nc.scalar.dma_start(
                    out=out_d[r0 + 128 : L, :], in_=ob[: L - r0 - 128, D:]
                )

        # ---- pipelined emission ----
        front(0)
        # img zero-fills (Pool/SWDGE): 4 slabs, once each; non-band cells
        # stay zero across reuses since every write hits the same cells
        for s in range(NSLAB):
            nc.gpsimd.dma_start(
                out=AP(tensor=img64.tensor, offset=s * 128 * SK2,
                       ap=[[SK2, 128], [1, SK2]]),
                in_=zt[:, :],
            )
        nc.gpsimd.dma_start(
            out=AP(tensor=img64.tensor, offset=NSLAB * 128 * SK2,
                   ap=[[SK2, 6], [1, SK2]]),
            in_=zt[:6, :],
        )
        # et tail cols: read only for invalid outputs l >= L; keep finite.
        # den tail (cols >= S): 1.0 so invalid rows' reciprocal stays finite
        nc.vector.memset(et[:, S - 6 :], 0.0)
        nc.vector.memset(den[:, S:], 1.0)
        front(1)
        shifts_part(0)  # et[:, 0:512); needs e cols [0, 512+6)
        t_chunks(0, 4)
        dma1p(0)
        dma1p(1)
        readp(0)
        front(2)
        shifts_part(1)
        t_chunks(4, 8)
        dma1p(2)
        readp(1)
        dma1p(3)
        dup_guard(6)
        readp(2)
        front(3)
        shifts_part(2)
        t_chunks(8, 12)
        dma1p(4)
        readp(3)
        dma1p(5)
        readp(4)
        t_chunks(12, 16)
        dma1p(6)
        dup_guard(12)
        readp(5)
        dma1p(7)
        for q in range(8):
            readp(q + 6)
            run_pair(q)

    nc.compile()
    return nc


_CACHE = {}


def _get_program():
    if "nc" not in _CACHE:
        _CACHE["nc"] = build_program()
    return _CACHE["nc"]


def kernel(x, W, b):
    x = np.asarray(x, dtype=np.float32)
    assert x.shape == (B, S, D), x.shape

    nc = _get_program()
    consts = _host_constants(W, b)
    in_maps = []
    for core in range(B):
        in_maps.append(
            {
                "x": np.ascontiguousarray(x[core]),
                "consts": consts,
            }
        )
    res = bass_utils.run_bass_kernel_spmd(nc, in_maps, core_ids=list(range(B)))
    out = np.stack([res.results[core]["out"] for core in range(B)], axis=0)
    return out


# revision 84
# speedup vs baseline: 1.5747x; 1.0172x over previous
"""Dynamic lightweight convolution TRN2 kernel — banded-matmul design.

out[b,l,d] = (1/K) * sum_k softmax_k(x[b,l+K-1,:] @ W + bias)[k, d%H] * x[b,l+k,d]

B=8, S=2048, D=1024, K=7, H=16, L=S-K+1=2042.
Sharding: data-parallel over batch, one batch element per NeuronCore (8 cores).

Per-core plan — the conv runs on the tensor engine as banded-matrix matmuls:

  1. x is loaded by GPSIMD (SWDGE) casting DMAs: f32 HBM -> bf16 SBUF, in
     graduated batches (chunks 0,1 alone for fast pipeline start, then
     2-chunk/3-chunk batches to amortize ~1us SWDGE descriptor-gen each).
  2. Logits path: PE-transpose x chunks -> xT, logits = W^T @ xT (PE, fp32
     psum), e = exp(logits + bias) (ACT), denominators via a [112,112]
     selector matmul (PE), rinv = 1/. (DVE), en = e * rinv (DVE).  W/bias
     columns are host-permuted k -> 6-k, so row 16j+h of en is the
     normalized weight of tap k = 6-j.
  3. Shifted SBUF->SBUF DMAs build et[16j+h, s] = en[16j+h, s+j] (one DMA
     per j, batched in 3 parts gated by front(1..3)); PE-transposes give
     T[s, r] (r = 16j+h) in t_all.
  4. Band construction via a DRAM bounce (the BIR verifier rejects SBUF
     dst APs with non-partition-exact steps, so the skew must happen on
     DRAM-side strides): T chunk pairs are written to zero-filled DRAM
     images at skewed offsets SKEW*p + r and read back with row pitch
     PITCH = SKEW-16, which lands T[p, r] at band position (p, 16p + r).
     Images are paired (2 tensors x 2 slabs) so one write covers 2 chunks
     and one read covers 2 blocks — halves the per-DMA issue overhead.
  5. Conv for 128-row output block b: for each h, a banded matmul
       out[l, d'] = sum_s A1_h[s, l] * x[s, 16d'+h]
     with stationary A1_h = a1[:, h : h+2048 : 16].  The 6-row contraction
     tail (s in the next chunk) uses a2: a tiny [6, 112] load from the
     next image's left guard + a second matmul into the same psum.
  6. psum [128, 1024] (h-major) -> SBUF staging with a de-interleaving
     copy; two blocks share one staging tile and one paired store DMA.

Queue plan (per-DMA SEQ occupancy ~650ns + head-of-line waits, so queues
are balanced and ordered by data readiness):
  SP (sync):   consts, paired dma1 band writes, paired a1 band reads
  Act (scalar): et shifts, output stores (+ exp, half the ob copies)
  Pool (gpsimd): input loads, zero fills, a2 guard loads (SWDGE)
  DVE (vector): xT copies, t_all copies, recip/mul, half the ob copies
"""

import numpy as np
import ml_dtypes
from contextlib import ExitStack

import concourse.bacc as bacc
import concourse.tile as tile
from concourse import mybir
from concourse import bass_utils
from concourse.ap import AP

K = 7
H = 16
B, S, D = 8, 2048, 1024
L = S - K + 1  # 2042
C = D // 128  # 8 d-chunks
NCH = S // 128  # 16 s-chunks
NB = 16  # output blocks of 128 rows (last has 122 valid)
KH = K * H  # 112

SLOT0 = 96  # img col of (l_rel=0, h=0): band windows are read from here
RCOLS = 1024  # band cols per 64-row sub-block window (16 per output row)
SK2 = 1152  # image write pitch: cell (s, 16j+h) lives at s*SK2 + 16j + h
P2 = SK2 - 16  # image read pitch: the -16/row slip forms the banded layout
NSLAB = 6  # rotating 128-row slabs, contiguous so reads span boundaries
IMG64_ELEMS = (NSLAB * 128 + 6) * SK2  # + 6-row wraparound guard

F32 = mybir.dt.float32
BF16 = mybir.dt.bfloat16

# byte offsets (per partition) inside the packed constants blob
_OFF_BIAS = 0      # [112, 1] f32
_OFF_IDENTB = 4    # [128, 128] bf16
_OFF_SELSUM = 260  # [112, 112] bf16
_OFF_WT = 484      # [128, 8, 112] bf16
_CONST_BYTES = 2276  # 569 f32 columns


def _host_constants(W, b):
    """Pack bias/identb/selsum/W into one [128, 569] f32 blob."""
    buf = np.zeros((128, _CONST_BYTES), np.uint8)

    def put(off, arr):
        by = np.ascontiguousarray(arr).view(np.uint8).reshape(arr.shape[0], -1)
        buf[: arr.shape[0], off : off + by.shape[1]] = by

    # Permute the k-axis (k -> 6-k) of W and bias so that logits/e/en rows
    # come out in j-order (row 16j+h is the weight for tap k=6-j), matching
    # the band-image run layout r = 16j+h.
    perm = np.array([16 * (K - 1 - j) + h for j in range(K) for h in range(H)])
    put(_OFF_BIAS, np.asarray(b, np.float32)[perm].reshape(KH, 1))
    put(_OFF_IDENTB, np.eye(128).astype(ml_dtypes.bfloat16))
    hh = np.arange(KH) % H
    selsum = ((hh[:, None] == hh[None, :]) * float(K)).astype(ml_dtypes.bfloat16)
    put(_OFF_SELSUM, selsum)
    # W [D, KH] -> permuted -> [128, C, KH] chunks (d = c*128 + p)
    wt = np.asarray(W, np.float32)[:, perm].astype(ml_dtypes.bfloat16)
    wt = wt.reshape(C, 128, KH).transpose(1, 0, 2).reshape(128, C * KH)
    put(_OFF_WT, np.ascontiguousarray(wt))
    return buf.view(np.float32)


def build_program():
    nc = bacc.Bacc(
        "TRN2", target_bir_lowering=False, debug=False, enable_asserts=True
    )

    x_d = nc.dram_tensor("x", [S, D], F32, kind="ExternalInput").ap()
    consts_d = nc.dram_tensor(
        "consts", [128, _CONST_BYTES // 4], F32, kind="ExternalInput"
    ).ap()
    out_d = nc.dram_tensor("out", [L, D], F32, kind="ExternalOutput").ap()
    # one contiguous band image: NSLAB rotating 128-row slabs + a 6-row
    # guard; chunk i writes slab i%NSLAB, and the 64-row sub-block reads
    # spill across slab boundaries (which is why slabs must be contiguous)
    img64 = nc.dram_tensor("img64", [IMG64_ELEMS], BF16, kind="Internal").ap()

    with tile.TileContext(nc) as tc, ExitStack() as ctx:
        singles = ctx.enter_context(tc.tile_pool(name="singles", bufs=1))
        xT_pool = ctx.enter_context(tc.tile_pool(name="xT", bufs=3))
        a1_pool = ctx.enter_context(tc.tile_pool(name="a1", bufs=3))
        outs_pool = ctx.enter_context(tc.tile_pool(name="outs", bufs=3))

        p_tp = ctx.enter_context(tc.tile_pool(name="ptp", bufs=2, space="PSUM"))
        p_log = ctx.enter_context(tc.tile_pool(name="plog", bufs=1, space="PSUM"))
        p_den = ctx.enter_context(tc.tile_pool(name="pden", bufs=1, space="PSUM"))
        p_out = ctx.enter_context(tc.tile_pool(name="pout", bufs=2, space="PSUM"))

        # ---- constants: identb first (gates the very first PE transpose),
        # then the rest of the packed blob ----
        cblob = singles.tile([128, _CONST_BYTES // 4], F32)
        cbytes = cblob.bitcast(mybir.dt.uint8)
        nc.sync.dma_start(
            out=cbytes[:, _OFF_IDENTB : _OFF_IDENTB + 256].bitcast(F32),
            in_=AP(tensor=consts_d.tensor, offset=_OFF_IDENTB // 4,
                   ap=[[_CONST_BYTES // 4, 128], [1, 64]]),
        )
        nc.sync.dma_start(
            out=cbytes[:, _OFF_SELSUM:].bitcast(F32),
            in_=AP(tensor=consts_d.tensor, offset=_OFF_SELSUM // 4,
                   ap=[[_CONST_BYTES // 4, 128],
                       [1, (_CONST_BYTES - _OFF_SELSUM) // 4]]),
        )
        nc.sync.dma_start(
            out=cbytes[:, :_OFF_IDENTB].bitcast(F32),
            in_=AP(tensor=consts_d.tensor, offset=0,
                   ap=[[_CONST_BYTES // 4, 128], [1, _OFF_IDENTB // 4]]),
        )

        def cview(off, nbytes, dt, rows=128):
            return cbytes[:rows, off : off + nbytes].bitcast(dt)

        bias_t = cview(_OFF_BIAS, 4, F32, rows=KH)
        identb_t = cview(_OFF_IDENTB, 256, BF16)
        selsum_t = cview(_OFF_SELSUM, 224, BF16, rows=KH)
        wt = cview(_OFF_WT, 1792, BF16).rearrange("p (c n) -> p c n", c=C)

        # GPSIMD ucode warmup — dependency-free so input DMAs start at t~0
        warm = singles.tile([1, 8], BF16)
        nc.gpsimd.memset(warm, 0.0)

        # PE p-state warmup: the cost model runs matmuls at half clock until
        # the engine has been continuously busy for 3us.  Burn the ramp on
        # dummy matmuls while the first input chunk is still in flight.
        junk = singles.tile([128, 128], BF16)
        nc.vector.memset(junk, 0.0)
        pwarm = p_out.tile([128, D], F32, tag="pout")
        for _ in range(44):
            nc.tensor.matmul(pwarm[:, :128], junk, junk, start=True, stop=True)

        # ---- persistent tensors ----
        xq = [
            singles.tile([128, 4 * D], BF16, name=f"xq{i}") for i in range(4)
        ]

        def xbv(i):
            return xq[i // 4][:, D * (i % 4) : D * (i % 4 + 1)]

        e_full = singles.tile([KH, S], BF16)
        et = singles.tile([KH, S], BF16)  # et[16j+h, s] = e[16j+h, s+j]
        den = singles.tile([16, S + 8], BF16)  # K * sum_k e[16k+h, s]
        rallo = singles.tile([128, NB, 16], F32)  # 1/den[h, l+6], l-major
        t_all = singles.tile([128, NCH, KH], BF16)  # T[s, r], chunked
        zt = singles.tile([128, SK2], BF16)  # zeros for image fill

        # ---- prologue ----
        # zero tile on DVE: fits in DVE's idle window before the first xT
        # copy, and keeps Pool free for input descriptor generation
        nc.vector.memset(zt, 0.0)
        # input DMAs gate the whole front pipeline;
        # graduated batching: chunks 0,1 alone so front(0) starts ASAP
        for lo, hi in ((0, 1), (1, 2), (2, 4), (4, 6), (6, 8), (8, 10),
                       (10, 12), (12, 16)):
            q, c0 = lo // 4, lo % 4
            assert hi - lo <= 4 and (hi - 1) // 4 == q
            nc.gpsimd.dma_start(
                out=AP(tensor=xq[q][:, :].tensor, offset=c0 * D,
                       ap=[[4 * D, 128], [D, hi - lo], [1, D]]),
                in_=AP(tensor=x_d.tensor, offset=128 * lo * D,
                       ap=[[D, 128], [128 * D, hi - lo], [1, D]]),
            )

        # ---- stage helpers ----
        def front(sb):
            """Transpose chunks 4sb..4sb+3, logits, exp, denom, rinv, en."""
            sl = slice(512 * sb, 512 * (sb + 1))
            xTt = xT_pool.tile([128, C, 512], BF16, tag="xT")
            for q in range(4):
                i = 4 * sb + q
                xbi = xbv(i)
                ptp = p_tp.tile([128, D], BF16, tag="ptp")
                for c in range(C):
                    nc.tensor.transpose(
                        ptp[:, 128 * c : 128 * (c + 1)],
                        xbi[:, 128 * c : 128 * (c + 1)],
                        identb_t,
                    )
                cp = nc.vector.tensor_copy if True else nc.scalar.copy
                cp(
                    xTt[:, :, 128 * q : 128 * (q + 1)],
                    ptp.rearrange("p (c s) -> p c s", c=C),
                )
            plog = p_log.tile([KH, 512], F32, tag="plog")
            for q in range(4):
                for c in range(C):
                    nc.tensor.matmul(
                        plog[:, 128 * q : 128 * (q + 1)],
                        wt[:, c, :],
                        xTt[:, c, 128 * q : 128 * (q + 1)],
                        start=(c == 0), stop=(c == C - 1),
                    )
            nc.scalar.activation(
                e_full[:, sl], plog,
                mybir.ActivationFunctionType.Exp, bias=bias_t, scale=1.0,
            )
            # denominators only (normalization is folded into the t_all
            # copy as a reciprocal multiply — keeps the shift DMAs and the
            # band pipeline off the recip/mul critical path)
            psd = p_log.tile([16, 512], F32, tag="plog")
            nc.tensor.matmul(
                psd, selsum_t[:, :16], e_full[:, sl], start=True, stop=True
            )
            nc.vector.tensor_copy(den[:, sl], psd)

        def shifts_part(part):
            """et[16j+h, s] = e[16j+h, s+j] — one DMA per j, three batches.

            Engine copies can't start at partition 16j (BIR rule: starts must
            be 0/32/64/96) and SBUF DMA APs need partition-exact steps, so:
            plain 2-dim SBUF->SBUF DMAs, one per j-group.  Part p covers the
            et cols ready after front(p+1): [0,512), [512,1024), [1024,2048).
            Sourcing raw e (not normalized weights) means these only wait on
            exp, not on the reciprocal chain.
            """
            c0 = 512 * part
            for j in range(K):
                ln = 512 if part < 2 else 1024 - j
                nc.scalar.dma_start(
                    out=AP(tensor=et[:, :].tensor, offset=16 * j * S + c0,
                           ap=[[S, 16], [1, ln]]),
                    in_=AP(tensor=e_full[:, :].tensor,
                           offset=16 * j * S + c0 + j,
                           ap=[[S, 16], [1, ln]]),
                )

        def t_chunks(lo, hi):
            for i in range(lo, hi):
                pt = p_tp.tile([128, D], BF16, tag="ptp")
                nc.tensor.transpose(
                    pt[:, :KH], et[:, 128 * i : 128 * (i + 1)],
                    identb_t[:KH, :KH],
                )
                nc.vector.tensor_copy(t_all[:, i, :], pt[:, :KH])

        def rall_block(b):
            """1/den for block b's output rows: rallo[l_rel, b, h] =
            1/den[h, 128b + l_rel + 6].  The band carries raw exp values;
            the softmax denominator (indexed at l+6) is applied to the
            output, folded into the de-interleave as a multiply."""
            pd = p_den.tile([128, 16], BF16, tag="pden")
            nc.tensor.transpose(
                pd, den[:, 128 * b + 6 : 128 * b + 134], identb_t[:16, :16]
            )
            nc.vector.reciprocal(rallo[:, b, :], pd)

        dma1_done = set()

        def dma1p(u):
            """T chunks 2u, 2u+1 -> adjacent band-image slabs (skewed)."""
            if u in dma1_done or u >= NB // 2:
                return
            dma1_done.add(u)
            nc.sync.dma_start(
                out=AP(tensor=img64.tensor,
                       offset=(2 * u % NSLAB) * 128 * SK2,
                       ap=[[SK2, 128], [128 * SK2, 2], [1, KH]]),
                in_=AP(tensor=t_all[:, :, :].tensor, offset=2 * u * KH,
                       ap=[[NCH * KH, 128], [KH, 2], [1, KH]]),
            )

        def dup_guard(i):
            """Duplicate chunk i's first 6 T-rows into the wraparound guard.

            Chunk i-1's sub-block-1 read spills 6 rows past slab NSLAB-1;
            when i % NSLAB == 0 those cells physically live in the guard
            instead of slab 0, so write them twice (idempotent values)."""
            nc.sync.dma_start(
                out=AP(tensor=img64.tensor, offset=NSLAB * 128 * SK2,
                       ap=[[SK2, 6], [1, KH]]),
                in_=AP(tensor=t_all[:, :, :].tensor, offset=i * KH,
                       ap=[[NCH * KH, 6], [1, KH]]),
            )

        a1_tiles = {}

        def readp(u):
            """Band reads for chunks 2u, 2u+1 -> one [128, 2*2048] tile.

            PE matmuls need stationary and moving operands on the same base
            partitions, so the three window pieces land where the x rows
            live: sub-0 window at partitions 0:70, sub-1 main at 64:128,
            sub-1 tail (chunk b+1 rows, crossing the slab boundary — slabs
            contiguous) at 0:6.  Chunk-paired 3-dim DMAs: 3 issues/pair.
            """
            if u in a1_tiles or u >= NB // 2:
                return
            a1 = a1_pool.tile([128, 2 * 2048], BF16, tag="a1")
            beta0 = (2 * u % NSLAB) * 128 * SK2 + SLOT0
            beta1 = beta0 + 64 * SK2
            nc.scalar.dma_start(
                out=AP(tensor=a1[:, :].tensor, offset=0,
                       ap=[[4096, 70], [2048, 2], [1, RCOLS]]),
                in_=AP(tensor=img64.tensor, offset=beta0,
                       ap=[[P2, 70], [128 * SK2, 2], [1, RCOLS]]),
            )
            nc.scalar.dma_start(
                out=AP(tensor=a1[:, :].tensor, offset=64 * 4096 + RCOLS,
                       ap=[[4096, 64], [2048, 2], [1, RCOLS]]),
                in_=AP(tensor=img64.tensor, offset=beta1,
                       ap=[[P2, 64], [128 * SK2, 2], [1, RCOLS]]),
            )
            nc.scalar.dma_start(
                out=AP(tensor=a1[:, :].tensor, offset=RCOLS,
                       ap=[[4096, 6], [2048, 2], [1, RCOLS]]),
                in_=AP(tensor=img64.tensor, offset=beta1 + 64 * P2,
                       ap=[[P2, 6], [128 * SK2, 2], [1, RCOLS]]),
            )
            a1_tiles[u] = a1

        def block(b, ob, obhalf):
            """Banded conv for output rows 128b .. 128b+nl -> staging tile.

            Two 64-row sub-blocks per chunk: sub 0's 70-row s-window sits
            inside chunk b (single matmul per h); sub 1 splits into a 64-row
            main (chunk b rows 64:128) plus a 6-row tail from chunk b+1.
            Matmul partition bases 0/64 are legal (0/32/64/96 rule)."""
            readp(b // 2)
            a1 = a1_tiles[b // 2]
            if b % 2 == 1:
                a1_tiles.pop(b // 2)
            rall_block(b)
            cb = (b % 2) * 2048
            xbb = xbv(b)
            xbn = xbv(b + 1) if b + 1 < NB else None
            po = p_out.tile([128, D], F32, tag="pout")
            for h in range(H):
                cl = slice(64 * h, 64 * (h + 1))
                nc.tensor.matmul(
                    po[:64, cl], a1[0:70, cb + h : cb + h + 16 * 63 + 1 : 16],
                    xbb[0:70, h :: H],
                    start=True, stop=True,
                )
                c1 = cb + RCOLS + h
                nc.tensor.matmul(
                    po[64:, cl], a1[64:128, c1 : c1 + 16 * 63 + 1 : 16],
                    xbb[64:128, h :: H],
                    start=True, stop=(b == NB - 1),
                )
                if b + 1 < NB:
                    nc.tensor.matmul(
                        po[64:, cl], a1[0:6, c1 : c1 + 16 * 63 + 1 : 16],
                        xbn[0:6, h :: H],
                        start=False, stop=True,
                    )
            # de-interleave h-major psum into natural channel order, folding
            # in the softmax denominator (broadcast over the 64 d' lanes)
            nc.vector.tensor_mul(
                ob[:, 1024 * obhalf : 1024 * (obhalf + 1)].rearrange(
                    "p (dp h) -> p h dp", h=H
                ),
                po.rearrange("p (h dp) -> p h dp", h=H),
                AP(tensor=rallo[:, :, :].tensor, offset=16 * b,
                   ap=[[NB * 16, 128], [1, 16], [0, 64]]),
            )

        def run_pair(q):
            """Blocks 2q, 2q+1 -> one staging tile -> one (or two) stores."""
            ob = outs_pool.tile([128, 2 * D], F32, tag="outs")
            block(2 * q, ob, 0)
            block(2 * q + 1, ob, 1)
            r0 = 256 * q
            if q < 7:
                nc.gpsimd.dma_start(
                    out=AP(tensor=out_d.tensor, offset=r0 * D,
                           ap=[[D, 128], [128 * D, 2], [1, D]]),
                    in_=AP(tensor=ob[:, :].tensor, offset=0,
                           ap=[[2 * D, 128], [D, 2], [1, D]]),
                )
            else:
                nc.gpsimd.dma_start(
                    out=out_d[r0 : r0 + 128, :], in_=ob[:, :D]
                )
                nc.gpsimd.dma_start(
                    out=out_d[r0 + 128 : L, :], in_=ob[: L - r0 - 128, D:]
                )

        # ---- pipelined emission ----
        front(0)
        # img zero-fills (Pool/SWDGE): 4 slabs, once each; non-band cells
        # stay zero across reuses since every write hits the same cells
        for s in range(NSLAB):
            nc.gpsimd.dma_start(
                out=AP(tensor=img64.tensor, offset=s * 128 * SK2,
                       ap=[[SK2, 128], [1, SK2]]),
                in_=zt[:, :],
            )
        nc.gpsimd.dma_start(
            out=AP(tensor=img64.tensor, offset=NSLAB * 128 * SK2,
                   ap=[[SK2, 6], [1, SK2]]),
            in_=zt[:6, :],
        )
        # et tail cols: read only for invalid outputs l >= L; keep finite.
        # den tail (cols >= S): 1.0 so invalid rows' reciprocal stays finite
        nc.vector.memset(et[:, S - 6 :], 0.0)
        nc.vector.memset(den[:, S:], 1.0)
        front(1)
        shifts_part(0)  # et[:, 0:512); needs e cols [0, 512+6)
        t_chunks(0, 4)
        dma1p(0)
        dma1p(1)
        readp(0)
        front(2)
        shifts_part(1)
        t_chunks(4, 8)
        dma1p(2)
        readp(1)
        dma1p(3)
        dup_guard(6)
        readp(2)
        front(3)
        shifts_part(2)
        t_chunks(8, 12)
        dma1p(4)
        readp(3)
        dma1p(5)
        readp(4)
        t_chunks(12, 16)
        dma1p(6)
        dup_guard(12)
        readp(5)
        dma1p(7)
        for q in range(8):
            readp(q + 6)
            run_pair(q)

    nc.compile()
    return nc


_CACHE = {}


def _get_program():
    if "nc" not in _CACHE:
        _CACHE["nc"] = build_program()
    return _CACHE["nc"]


def kernel(x, W, b):
    x = np.asarray(x, dtype=np.float32)
    assert x.shape == (B, S, D), x.shape

    nc = _get_program()
    consts = _host_constants(W, b)
    in_maps = []
    for core in range(B):
        in_maps.append(
            {
                "x": np.ascontiguousarray(x[core]),
                "consts": consts,
            }
        )
    res = bass_utils.run_bass_kernel_spmd(nc, in_maps, core_ids=list(range(B)))
    out = np.stack([res.results[core]["out"] for core in range(B)], axis=0)
    return out
